# revision 2
# baseline (speedup 1.0000x reference)
"""CapsuleLayer (dynamic routing) on 8 trn2 NeuronCores.

Math: u_hat[b,c,i,o] = sum_{d,k} W[c,0,i,o,d,k] x[b,i,k]
             = sum_k Wsum[c,i,o,k] x[b,i,k],  Wsum = W.sum(d)   (134MB -> 8.4MB)
Routing logits are cumulative: b_t = u_hat . (sum_{tau<t} v_tau), so each
iteration only needs the running vector-sum w.  Everything is sharded over
IN_CAPS (i) across 8 cores; only s[b,c,o] (131KB) crosses cores, reduced on
host between launches.

Launch 1 (per core, i-slice of 256):
  - reduce W over d -> Wsum, stored to HBM in two layouts:
      wa16 [c,(i k),o] (bf16, s-matmul lhsT) and wb2 [q,c,o,128] (f32, G lhsT)
  - transpose x -> xt [(i k), b] (bf16)
  - s0_partial[c,o,b] = sum_{ik in slice} Wsum^T x   (uniform-c iteration 0)
Launch 2/3 (same kernel, different w input):
  - G = Wsum^T_o w  (PE), P = x*G (DVE), b_t = sum_k P (PE E-fold)
  - softmax over classes -> c_t
  - crep = k-replicate(c_t) (PE), y = x*crep (DVE), s_part = Wsum^T y (PE)
Host: s = sum over cores, v = squash(s), w accumulates v.
"""

import contextlib
import sys
import types

import numpy as np
import ml_dtypes  # noqa: F401  (bf16 array dtype for I/O maps)


def _install_ntff_shim():
    """The image's antenv lacks axon_hooks; provide a minimal equivalent so
    run_bass_kernel_spmd(trace=True) can capture NTFF profiles via the
    injected libaxon_pjrt.so.  No-op if the real module exists or the .so
    is unavailable (grading path uses trace=False and never hits this)."""
    try:
        import antenv.axon_hooks  # noqa: F401

        return
    except Exception:
        pass
    import ctypes

    mod = types.ModuleType("antenv.axon_hooks")
    holder = [None, False]

    def set_axon_ntff_profile_hook(h):
        holder[0], holder[1] = h, True

    def _make_hook():
        try:
            lib = ctypes.CDLL("/opt/axon/libaxon_pjrt.so")
        except OSError:
            return None
        if not hasattr(lib, "axon_start_nrt_profile"):
            return None
        lib.axon_start_nrt_profile.argtypes = [
            ctypes.POINTER(ctypes.c_int64),
            ctypes.c_size_t,
        ]
        lib.axon_start_nrt_profile.restype = ctypes.c_int64
        lib.axon_stop_nrt_profile.argtypes = [ctypes.c_char_p]
        lib.axon_stop_nrt_profile.restype = ctypes.c_int64

        @contextlib.contextmanager
        def _hook(output_dir, device_ids):
            import jax

            jax.devices()
            if device_ids:
                ids = (ctypes.c_int64 * len(device_ids))(*device_ids)
                rc = lib.axon_start_nrt_profile(ids, len(device_ids))
            else:
                rc = lib.axon_start_nrt_profile(None, 0)
            if rc != 0:
                raise RuntimeError(f"axon_start_nrt_profile rc={rc}")
            try:
                yield
            finally:
                n = lib.axon_stop_nrt_profile(str(output_dir).encode())
                print(
                    f"profile: {n} file(s) written to {output_dir}",
                    file=sys.stderr,
                )

        return _hook

    def get_axon_ntff_profile_hook():
        if not holder[1]:
            holder[0], holder[1] = _make_hook(), True
        return holder[0]

    mod.set_axon_ntff_profile_hook = set_axon_ntff_profile_hook
    mod.get_axon_ntff_profile_hook = get_axon_ntff_profile_hook
    sys.modules["antenv.axon_hooks"] = mod


try:
    _install_ntff_shim()
except Exception:
    pass

import concourse.bass as bass
import concourse.mybir as mybir
import concourse.tile as tile
from concourse import masks
from concourse.bass_utils import run_bass_kernel_spmd
from bass_rust import ScopedClock

# ---------------------------------------------------------------- constants
C, I, O, D, K, B = 8, 2048, 16, 16, 8, 256
NCORES = 8
IL = I // NCORES          # 256 i's per core
CH = IL * K // 128        # 16 (i,k)-chunks of 128 partitions per core
F32 = mybir.dt.float32
F32R = mybir.dt.float32r
BF16 = mybir.dt.bfloat16
CB = C * B

# ------------------------------------------------- tile tail-drain workaround
_MAX_WAITS = 1


def _patched_drain_and_barrier(self, tick_clock, wait_clock):
    nc = self.nc
    drain_inst = nc.sync.drain()
    wait_clock.add_sem_waits(
        drain_inst.ins, ScopedClock({None: tick_clock.global_clock})
    )
    si = drain_inst.ins.sync_info
    if si is not None and si.on_wait and len(si.on_wait) > _MAX_WAITS:
        waits = list(si.on_wait)
        si.on_wait = waits[:_MAX_WAITS]
        for i in range(_MAX_WAITS, len(waits), _MAX_WAITS):
            extra = nc.sync.drain()
            extra.ins.sync_info = mybir.SyncInfo(
                on_wait=waits[i : i + _MAX_WAITS], on_update=[]
            )
    nc.all_engine_barrier()
    assert self.sems is not None
    popped = nc._tile_sem_poison_stack.pop()
    assert popped is self._sem_poison
    nc.clear_and_free_semaphores(list(self.sems.allocated().values()))
    nc.all_engine_barrier()


tile.TileContext._drain_and_barrier = _patched_drain_and_barrier

_fix_ctr = [0]


def fixup_multi_waits(nc):
    """walrus in this toolchain accepts at most one sem wait per instruction;
    hoist extra waits onto same-engine drains placed just before."""
    for f in nc.m.functions:
        for bb in f.blocks:
            out = []
            for inst in bb.instructions:
                si = inst.sync_info
                if si is not None and si.on_wait and len(si.on_wait) > _MAX_WAITS:
                    waits = list(si.on_wait)
                    for i in range(0, len(waits) - _MAX_WAITS, _MAX_WAITS):
                        _fix_ctr[0] += 1
                        d = mybir.InstDrain(
                            name=f"waitsplit_{_fix_ctr[0]}", ins=[], outs=[]
                        )
                        d.engine = inst.engine
                        d.sync_info = mybir.SyncInfo(
                            on_wait=waits[i : i + _MAX_WAITS], on_update=[]
                        )
                        out.append(d)
                    si.on_wait = waits[len(waits) - _MAX_WAITS :]
                out.append(inst)
            bb.instructions[:] = out
    return nc



def build_all(fixup=True):
    nc = bass.Bass("TRN2", target_bir_lowering=False, debug=False,
                   num_devices=NCORES)
    W_d = nc.dram_tensor("W", [C, IL, O, D, K], F32, kind="ExternalInput").ap()
    x_d = nc.dram_tensor("x", [B, IL, K], F32, kind="ExternalInput").ap()
    v_d = nc.dram_tensor("v", [C, O, B], F32R, kind="ExternalOutput").ap()
    wa16_d = nc.dram_tensor("wa16", [C, IL * K, O], BF16).ap()
    wb2_d = nc.dram_tensor("wb2", [CH, C, O, 128], BF16).ap()
    # collective bounce buffers (one pair per iteration)
    cc_in = [nc.dram_tensor(f"cc_in{t}", [16, CB], F32).ap() for t in range(3)]
    cc_out = [nc.dram_tensor(f"cc_out{t}", [16, CB], F32).ap() for t in range(3)]

    with tile.TileContext(nc) as tc:
        with (
            tc.tile_pool(name="const", bufs=1) as constp,
            tc.tile_pool(name="persist", bufs=1) as pers,
            tc.tile_pool(name="small", bufs=4) as smallp,
            tc.tile_pool(name="work", bufs=3) as workp,
            tc.tile_pool(name="soft", bufs=2) as softp,
            tc.tile_pool(name="sqpool", bufs=1) as sqp,
        ):
            # ---------------- constants
            ident = constp.tile([128, 128], F32)
            masks.make_identity(nc, ident[:])
            identb = constp.tile([128, 128], BF16)
            with nc.allow_low_precision(reason="identity copy"):
                nc.vector.tensor_copy(identb[:], ident[:])
            e_big = constp.tile([128, 256], BF16)
            nc.gpsimd.memset(e_big[:], 1.0)
            nc.gpsimd.affine_select(
                out=e_big[:], in_=e_big[:],
                compare_op=mybir.AluOpType.is_ge, fill=0.0,
                base=1024, pattern=[[-8, 256]], channel_multiplier=1)
            nc.gpsimd.affine_select(
                out=e_big[:], in_=e_big[:],
                compare_op=mybir.AluOpType.is_ge, fill=0.0,
                base=-1017, pattern=[[8, 256]], channel_multiplier=-1)
            e2_big = constp.tile([128, 1152], BF16)
            nc.gpsimd.memset(e2_big[:], 1.0)
            nc.gpsimd.affine_select(
                out=e2_big[:], in_=e2_big[:],
                compare_op=mybir.AluOpType.is_ge, fill=0.0,
                base=0, pattern=[[1, 1152]], channel_multiplier=-8)
            nc.gpsimd.affine_select(
                out=e2_big[:], in_=e2_big[:],
                compare_op=mybir.AluOpType.is_ge, fill=0.0,
                base=7, pattern=[[-1, 1152]], channel_multiplier=8)
            ones16f = constp.tile([16, 1], F32)
            nc.gpsimd.memset(ones16f[:], 1.0)
            ones16 = constp.tile([16, 1], F32R)
            ones1f = constp.tile([1, 16], F32)
            nc.gpsimd.memset(ones1f[:], 1.0)
            ones1 = constp.tile([1, 16], F32R)
            with nc.allow_low_precision(reason="ones copy"):
                nc.vector.tensor_copy(ones16[:], ones16f[:])
                nc.vector.tensor_copy(ones1[:], ones1f[:])

            # ---------------- persistent state
            xt16 = pers.tile([128, CH * B], BF16)
            wa_all = pers.tile([128, C * CH * O], BF16)
            w_acc = pers.tile([16, CB], BF16)
            bt_sb = pers.tile([128, 2 * CB], F32)
            ct_all = pers.tile([128, 2 * CB], BF16)

            # ---------------- phases A-C (scoped SBUF: xt32, W/x staging)
            phio_cm = contextlib.ExitStack()
            phio = phio_cm.enter_context(tc.tile_pool(name="phio", bufs=3))
            with tc.tile_pool(name="xps", bufs=4, space="PSUM") as xps:
                for bc in range(2):
                    xin = phio.tile([128, IL * K], F32, tag="xin", bufs=2)
                    nc.sync.dma_start(
                        xin[:],
                        x_d[bc * 128 : (bc + 1) * 128].rearrange("b i k -> b (i k)"),
                    )
                    for q in range(CH):
                        ps = xps.tile([128, 128], F32)
                        nc.tensor.transpose(
                            ps[:], xin[:, q * 128 : (q + 1) * 128], ident[:]
                        )
                        nc.scalar.copy(
                            xt16[:, q * B + bc * 128 : q * B + bc * 128 + 128],
                            ps[:],
                        )

            # ---------------- phase B: W reduce over d
            for t in range(2 * C):
                c, ih = t // 2, t % 2
                wt = phio.tile([128, O * D * K], F32, tag="wt", bufs=2)
                (nc.sync if t % 2 == 0 else nc.scalar).dma_start(
                    wt[:],
                    W_d[c, ih * 128 : (ih + 1) * 128].rearrange("p o d k -> p (o d k)"),
                )
                wf = smallp.tile([128, K * O], F32, tag="wf")
                nc.vector.reduce_sum(
                    wf[:].rearrange("p (k o) -> p o k", k=K),
                    wt[:].rearrange("p (o d k) -> p o k d", o=O, d=D, k=K),
                    axis=mybir.AxisListType.X,
                )
                wf16 = smallp.tile([128, K * O], BF16, tag="wf16")
                nc.vector.tensor_copy(wf16[:], wf[:])
                # flat contiguous write: dst row i <-> 128 els (k*16+o)
                dst16 = wa16_d[c].rearrange("(i f) o -> i (f o)", f=K)[
                    ih * 128 : (ih + 1) * 128
                ]
                nc.scalar.dma_start(dst16, wf16[:])

            # ---------------- phase C: round-trip -> s0 + wb2 + wa_all
            with (
                tc.tile_pool(name="tps", bufs=2, space="PSUM") as tpsp,
                tc.tile_pool(name="s0ps", bufs=2, space="PSUM") as s0ps,
            ):
                for c in range(C):
                    s0p = s0ps.tile([16, B], F32, tag="s0p")
                    tp = None
                    for q in range(CH):
                        wa = wa_all[:, c * CH * O + q * O : c * CH * O + (q + 1) * O]
                        (nc.sync if q % 2 else nc.scalar).dma_start(
                            wa, wa16_d[c, q * 128 : (q + 1) * 128]
                        )
                        nc.tensor.matmul(
                            s0p[:], wa, xt16[:, q * B : (q + 1) * B],
                            start=(q == 0), stop=(q == CH - 1),
                        )
                        if q % 4 == 0:
                            tp = tpsp.tile([16, 512], BF16, tag="tp",
                                           name=f"tp_{c}_{q}")
                        nc.tensor.transpose(
                            tp[:, (q % 4) * 128 : (q % 4) * 128 + 128],
                            wa, identb[:],
                        )
                        if q % 4 == 3:
                            wbp = phio.tile([16, 512], BF16, tag="wbp", bufs=2)
                            nc.scalar.copy(wbp[:], tp[:])
                            nc.scalar.dma_start(
                                wb2_d[q - 3 : q + 1, c].rearrange("q o s -> o q s"),
                                wbp[:].rearrange("o (q s) -> o q s", s=128),
                            )
                    s0sb = smallp.tile([16, B], F32, tag="s_sb", name=f"s0sb{c}")
                    nc.scalar.copy(s0sb[:], s0p[:])
                    nc.sync.dma_start(cc_in[0][:, c * B : (c + 1) * B], s0sb[:])

            # ---------------- allreduce + squash helper
            def allreduce_squash(t, pre, last):
                """cc_in[t] holds the local partial of s/pre; reduce, squash
                v = squash(pre * s_sum), accumulate into w_acc or emit v."""
                nc.gpsimd.collective_compute(
                    "AllReduce",
                    mybir.AluOpType.add,
                    replica_groups=[list(range(NCORES))],
                    ins=[cc_in[t].opt()],
                    outs=[cc_out[t].opt()],
                )
                s_sum = sqp.tile([16, CB], F32, tag="s_sum", name=f"s_sum{t}")
                nc.sync.dma_start(s_sum[:], cc_out[t][:, :])
                sq = sqp.tile([16, CB], F32R, tag="sq", name=f"sq{t}")
                nc.scalar.activation(
                    sq[:], s_sum[:], mybir.ActivationFunctionType.Square,
                    scale=pre,
                )
                with tc.tile_pool(name=f"sqps{t}", bufs=1, space="PSUM") as sqps:
                    ssq_ps = sqps.tile([1, CB], F32, tag="ssq")
                    for j in range(4):
                        nc.tensor.matmul(
                            ssq_ps[:, j * 512 : (j + 1) * 512],
                            ones16[:],
                            sq[:, j * 512 : (j + 1) * 512],
                            start=True, stop=True,
                        )
                    ssq_row = sqp.tile([1, CB], F32R, tag="row_tmp",
                                       name=f"ssq_row{t}")
                    nc.scalar.copy(ssq_row[:], ssq_ps[:])
                # reshape to [128, 16] for cheap elementwise math
                ssq = sqp.tile([128, 16], F32R, tag="ssq_rs", name=f"ssq_rs{t}")
                nc.sync.dma_start(
                    ssq[:], ssq_row[:].rearrange("u (p f) -> u p f", p=128)
                )
                den1 = sqp.tile([128, 16], F32, tag="den1", name=f"den1{t}")
                nc.vector.tensor_scalar_add(den1[:], ssq[:], 1.0)
                r1 = sqp.tile([128, 16], F32, tag="r1", name=f"r1{t}")
                nc.vector.reciprocal(r1[:], den1[:])
                rt = sqp.tile([128, 16], F32, tag="rt", name=f"rt{t}")
                nc.scalar.sqrt(rt[:], ssq[:])
                r2 = sqp.tile([128, 16], F32, tag="r2", name=f"r2{t}")
                nc.vector.reciprocal(r2[:], rt[:])
                m1 = sqp.tile([128, 16], F32, tag="m1", name=f"m1{t}")
                nc.vector.tensor_mul(m1[:], ssq[:], r1[:])
                scale_rs = sqp.tile([128, 16], F32R, tag="scale_rs",
                                    name=f"scale_rs{t}")
                nc.vector.tensor_mul(scale_rs[:], m1[:], r2[:])
                if pre != 1.0:
                    nc.vector.tensor_scalar_mul(scale_rs[:], scale_rs[:], pre)
                scale_row = sqp.tile([1, CB], F32R, tag="row_tmp",
                                     name=f"scale_row{t}")
                nc.sync.dma_start(
                    scale_row[:].rearrange("u (p f) -> u p f", p=128), scale_rs[:]
                )
                with tc.tile_pool(name=f"bcps{t}", bufs=1, space="PSUM") as bcps:
                    bc_ps = bcps.tile([16, CB], F32, tag="bc")
                    for j in range(4):
                        nc.tensor.matmul(
                            bc_ps[:, j * 512 : (j + 1) * 512],
                            ones1[:],
                            scale_row[:, j * 512 : (j + 1) * 512],
                            start=True, stop=True,
                        )
                    v_sb = sqp.tile([16, CB], F32R, tag="v_sbr",
                                    name=f"v_sbr{t}")
                    with nc.allow_low_precision(reason="f32r full range"):
                        nc.vector.tensor_mul(v_sb[:], s_sum[:], bc_ps[:])
                    if last:
                        for c in range(C):
                            nc.sync.dma_start(
                                v_d[c], v_sb[:, c * B : (c + 1) * B]
                            )
                    elif t == 0:
                        nc.vector.tensor_copy(w_acc[:], v_sb[:])
                    else:
                        with nc.allow_low_precision(reason="w accum"):
                            nc.vector.tensor_add(w_acc[:], w_acc[:], v_sb[:])

            phio_cm.close()

            allreduce_squash(0, 1.0 / C, last=False)

            # ---------------- routing iterations 1 and 2
            for it in range(1, 3):
                # phase 1: b_t
                with (
                    tc.tile_pool(name=f"btps{it}", bufs=1, space="PSUM") as btps,
                    tc.tile_pool(name=f"gps{it}", bufs=2, space="PSUM") as gps,
                ):
                    for h in range(2):
                        bt_ps = btps.tile([128, CB], F32, tag="bt")
                        for qq in range(CH // 2):
                            q = h * 8 + qq
                            wbq = workp.tile([16, C * 128], BF16, tag="wbq")
                            nc.sync.dma_start(
                                wbq[:].rearrange("o (c s) -> o c s", c=C),
                                wb2_d[q].rearrange("c o s -> o c s"),
                            )
                            p_sb = workp.tile([128, CB], BF16, tag="p_sb")
                            for piece in range(2):
                                g_ps = gps.tile([128, 1024], F32, tag="g")
                                for cc in range(4):
                                    c = piece * 4 + cc
                                    nc.tensor.matmul(
                                        g_ps[:, cc * B : (cc + 1) * B],
                                        wbq[:, c * 128 : (c + 1) * 128],
                                        w_acc[:, c * B : (c + 1) * B],
                                        start=True, stop=True,
                                    )
                                g_sb = workp.tile([128, 1024], BF16, tag="g_sb")
                                nc.scalar.copy(g_sb[:], g_ps[:])
                                xb = (
                                    xt16[:, q * B : (q + 1) * B]
                                    .rearrange("p (u b) -> p u b", u=1)
                                    .broadcast_to([128, 4, B])
                                )
                                nc.vector.tensor_mul(
                                    p_sb[:, piece * 1024 : (piece + 1) * 1024]
                                    .rearrange("p (c b) -> p c b", c=4),
                                    xb,
                                    g_sb[:].rearrange("p (c b) -> p c b", c=4),
                                )
                            eq = e_big[:, 128 - 16 * qq : 256 - 16 * qq]
                            for j in range(4):
                                nc.tensor.matmul(
                                    bt_ps[:, j * 512 : (j + 1) * 512],
                                    eq,
                                    p_sb[:, j * 512 : (j + 1) * 512],
                                    start=(qq == 0), stop=(qq == CH // 2 - 1),
                                )
                        nc.scalar.copy(bt_sb[:, h * CB : (h + 1) * CB], bt_ps[:])

                # phase 2: softmax over classes
                for h in range(2):
                    bt = bt_sb[:, h * CB : (h + 1) * CB]
                    rmax = softp.tile([128, B], F32, tag="rmax")
                    nc.vector.reduce_max(
                        rmax[:],
                        bt.rearrange("p (c b) -> p b c", c=C),
                        axis=mybir.AxisListType.X,
                    )
                    sub = sqp.tile([128, CB], F32, tag="sub")
                    nc.vector.tensor_sub(
                        sub[:].rearrange("p (c b) -> p c b", c=C),
                        bt.rearrange("p (c b) -> p c b", c=C),
                        rmax[:].rearrange("p (u b) -> p u b", u=1).broadcast_to(
                            [128, C, B]
                        ),
                    )
                    e_t = softp.tile([128, CB], BF16, tag="e_t")
                    nc.scalar.activation(
                        e_t[:], sub[:], mybir.ActivationFunctionType.Exp
                    )
                    den = softp.tile([128, B], F32, tag="den")
                    nc.vector.reduce_sum(
                        den[:],
                        e_t[:].rearrange("p (c b) -> p b c", c=C),
                        axis=mybir.AxisListType.X,
                    )
                    rec = softp.tile([128, B], F32, tag="rec")
                    nc.vector.reciprocal(rec[:], den[:])
                    nc.vector.tensor_mul(
                        ct_all[:, h * CB : (h + 1) * CB].rearrange(
                            "p (c b) -> p c b", c=C
                        ),
                        e_t[:].rearrange("p (c b) -> p c b", c=C),
                        rec[:].rearrange("p (u b) -> p u b", u=1).broadcast_to(
                            [128, C, B]
                        ),
                    )

                # phase 3+4: crep -> y -> s
                with (
                    tc.tile_pool(name=f"sps{it}", bufs=1, space="PSUM") as sps,
                    tc.tile_pool(name=f"crps{it}", bufs=2, space="PSUM") as crps,
                ):
                    for grp in range(2):
                        s_ps = [
                            sps.tile([16, B], F32, tag=f"s{cc}",
                                     name=f"s_ps{it}_{grp}_{cc}")
                            for cc in range(4)
                        ]
                        for q in range(CH):
                            h, qq = q // 8, q % 8
                            ct_half = ct_all[:, h * CB : (h + 1) * CB]
                            cr_ps = crps.tile([128, 1024], F32, tag="cr")
                            for j in range(2):
                                nc.tensor.matmul(
                                    cr_ps[:, j * 512 : (j + 1) * 512],
                                    e2_big[:, 128 * qq : 128 * qq + 128],
                                    ct_half[:, grp * 1024 + j * 512 :
                                            grp * 1024 + (j + 1) * 512],
                                    start=True, stop=True,
                                )
                            cr_sb = workp.tile([128, 1024], BF16, tag="cr_sb")
                            nc.scalar.copy(cr_sb[:], cr_ps[:])
                            xb = (
                                xt16[:, q * B : (q + 1) * B]
                                .rearrange("p (u b) -> p u b", u=1)
                                .broadcast_to([128, 4, B])
                            )
                            y_q = workp.tile([128, 1024], BF16, tag="y_q")
                            nc.vector.tensor_mul(
                                y_q[:].rearrange("p (c b) -> p c b", c=4),
                                xb,
                                cr_sb[:].rearrange("p (c b) -> p c b", c=4),
                            )
                            for cc in range(4):
                                c = grp * 4 + cc
                                nc.tensor.matmul(
                                    s_ps[cc][:],
                                    wa_all[:, c * CH * O + q * O :
                                           c * CH * O + (q + 1) * O],
                                    y_q[:, cc * B : (cc + 1) * B],
                                    start=(q == 0), stop=(q == CH - 1),
                                )
                        for cc in range(4):
                            c = grp * 4 + cc
                            s_sb = smallp.tile([16, B], F32, tag="s_sb",
                                               name=f"s_sb{it}_{c}")
                            nc.scalar.copy(s_sb[:], s_ps[cc][:])
                            nc.sync.dma_start(
                                cc_in[it][:, c * B : (c + 1) * B], s_sb[:]
                            )
                allreduce_squash(it, 1.0, last=(it == 2))
    return fixup_multi_waits(nc) if fixup else nc


_NC = None


def kernel(x: np.ndarray, W: np.ndarray, _timings=None) -> np.ndarray:
    global _NC
    x = np.asarray(x, np.float32)
    W = np.asarray(W, np.float32)
    if _NC is None:
        _NC = build_all()
    in_maps = []
    for j in range(NCORES):
        sl = slice(j * IL, (j + 1) * IL)
        in_maps.append(
            {
                "W": np.ascontiguousarray(W[:, 0, sl]),
                "x": np.ascontiguousarray(x[:, sl, :]),
            }
        )
    res = run_bass_kernel_spmd(
        _NC, in_maps, core_ids=list(range(NCORES)),
        trace=_timings is not None,
    )
    if _timings is not None:
        _timings.append(res.exec_time_ns)
    v = res.results[0]["v"].astype(np.float32)  # [C, O, B]
    return np.ascontiguousarray(v.transpose(2, 0, 1))



# revision 28
# speedup vs baseline: 1.0067x; 1.0067x over previous
"""CapsuleLayer (dynamic routing) on 8 trn2 NeuronCores.

Math: u_hat[b,c,i,o] = sum_{d,k} W[c,0,i,o,d,k] x[b,i,k]
             = sum_k Wsum[c,i,o,k] x[b,i,k],  Wsum = W.sum(d)   (134MB -> 8.4MB)
Routing logits are cumulative: b_t = u_hat . (sum_{tau<t} v_tau), so each
iteration only needs the running vector-sum w.  Everything is sharded over
IN_CAPS (i) across 8 cores; only s[b,c,o] (131KB) crosses cores via AllReduce.

Per-core layouts (partition dim = i throughout the routing iterations):
  xt_i[h]   [128(i), (k,b)]  bf16  - x transposed via PE + 1MB HBM round-trip
  wf        [128(i), (c,h,(k,o))] bf16 - Wsum, d-reduced on DVE/Pool trees
  T_all     [128(k,o), (c,h,i)]  bf16 - PE-transpose of wf (G stationaries)
  w_acc     [16(o), (c,b)]  bf16 - running sum of squash outputs v
Iteration t:
  G_k[i,b]  = T[c,h,k-slice]^T w_acc[c]          (PE, K=o=16)
  P         = xt_i * G (PSUM f32 read, DVE/Pool), bt = sum_k P (tree adds)
  c_t       = softmax_c(bt)  (exp on Act, den tree, no max-subtraction)
  y_c       = ct_c (bcast over k) * xt_i         (DVE, all bf16)
  s_c[o,b] += wf[c,h,k-slice]^T y_c              (PE, accumulate 16 matmuls)
AllReduce s (131KB f32) -> squash -> w_acc (or v output on last iter).
"""

import contextlib
import sys
import types

import numpy as np
import ml_dtypes  # noqa: F401  (bf16 array dtype for I/O maps)


def _install_ntff_shim():
    """The image's antenv lacks axon_hooks; provide a minimal equivalent so
    run_bass_kernel_spmd(trace=True) can capture NTFF profiles via the
    injected libaxon_pjrt.so.  No-op if the real module exists or the .so
    is unavailable (grading path uses trace=False and never hits this)."""
    try:
        import antenv.axon_hooks  # noqa: F401

        return
    except Exception:
        pass
    import ctypes

    mod = types.ModuleType("antenv.axon_hooks")
    holder = [None, False]

    def set_axon_ntff_profile_hook(h):
        holder[0], holder[1] = h, True

    def _make_hook():
        try:
            lib = ctypes.CDLL("/opt/axon/libaxon_pjrt.so")
        except OSError:
            return None
        if not hasattr(lib, "axon_start_nrt_profile"):
            return None
        lib.axon_start_nrt_profile.argtypes = [
            ctypes.POINTER(ctypes.c_int64),
            ctypes.c_size_t,
        ]
        lib.axon_start_nrt_profile.restype = ctypes.c_int64
        lib.axon_stop_nrt_profile.argtypes = [ctypes.c_char_p]
        lib.axon_stop_nrt_profile.restype = ctypes.c_int64

        @contextlib.contextmanager
        def _hook(output_dir, device_ids):
            import jax

            jax.devices()
            if device_ids:
                ids = (ctypes.c_int64 * len(device_ids))(*device_ids)
                rc = lib.axon_start_nrt_profile(ids, len(device_ids))
            else:
                rc = lib.axon_start_nrt_profile(None, 0)
            if rc != 0:
                raise RuntimeError(f"axon_start_nrt_profile rc={rc}")
            try:
                yield
            finally:
                n = lib.axon_stop_nrt_profile(str(output_dir).encode())
                print(
                    f"profile: {n} file(s) written to {output_dir}",
                    file=sys.stderr,
                )

        return _hook

    def get_axon_ntff_profile_hook():
        if not holder[1]:
            holder[0], holder[1] = _make_hook(), True
        return holder[0]

    mod.set_axon_ntff_profile_hook = set_axon_ntff_profile_hook
    mod.get_axon_ntff_profile_hook = get_axon_ntff_profile_hook
    sys.modules["antenv.axon_hooks"] = mod


try:
    _install_ntff_shim()
except Exception:
    pass

import concourse.bass as bass
import concourse.mybir as mybir
import concourse.tile as tile
from concourse import masks
from concourse.bass_utils import run_bass_kernel_spmd
from bass_rust import ScopedClock

# ---------------------------------------------------------------- constants
C, I, O, D, K, B = 8, 2048, 16, 16, 8, 256
NCORES = 8
IL = I // NCORES          # 256 i's per core
F32 = mybir.dt.float32
F32R = mybir.dt.float32r
BF16 = mybir.dt.bfloat16
CB = C * B
KB_ = K * B               # 2048

# ------------------------------------------------- tile tail-drain workaround
_MAX_WAITS = 1


def _patched_drain_and_barrier(self, tick_clock, wait_clock):
    nc = self.nc
    drain_inst = nc.sync.drain()
    wait_clock.add_sem_waits(
        drain_inst.ins, ScopedClock({None: tick_clock.global_clock})
    )
    si = drain_inst.ins.sync_info
    if si is not None and si.on_wait and len(si.on_wait) > _MAX_WAITS:
        waits = list(si.on_wait)
        si.on_wait = waits[:_MAX_WAITS]
        for i in range(_MAX_WAITS, len(waits), _MAX_WAITS):
            extra = nc.sync.drain()
            extra.ins.sync_info = mybir.SyncInfo(
                on_wait=waits[i : i + _MAX_WAITS], on_update=[]
            )
    nc.all_engine_barrier()
    assert self.sems is not None
    popped = nc._tile_sem_poison_stack.pop()
    assert popped is self._sem_poison
    nc.clear_and_free_semaphores(list(self.sems.allocated().values()))
    nc.all_engine_barrier()


tile.TileContext._drain_and_barrier = _patched_drain_and_barrier

_fix_ctr = [0]


def fixup_multi_waits(nc):
    """walrus in this toolchain accepts at most one sem wait per instruction;
    hoist extra waits onto same-engine drains placed just before."""
    for f in nc.m.functions:
        for bb in f.blocks:
            out = []
            for inst in bb.instructions:
                si = inst.sync_info
                if si is not None and si.on_wait and len(si.on_wait) > _MAX_WAITS:
                    waits = list(si.on_wait)
                    for i in range(0, len(waits) - _MAX_WAITS, _MAX_WAITS):
                        _fix_ctr[0] += 1
                        d = mybir.InstDrain(
                            name=f"waitsplit_{_fix_ctr[0]}", ins=[], outs=[]
                        )
                        d.engine = inst.engine
                        d.sync_info = mybir.SyncInfo(
                            on_wait=waits[i : i + _MAX_WAITS], on_update=[]
                        )
                        out.append(d)
                    si.on_wait = waits[len(waits) - _MAX_WAITS :]
                out.append(inst)
            bb.instructions[:] = out
    return nc


def build_all(fixup=True):
    nc = bass.Bass("TRN2", target_bir_lowering=False, debug=False,
                   num_devices=NCORES)
    W_d = nc.dram_tensor("W", [C, IL, O, D, K], F32, kind="ExternalInput").ap()
    x_d = nc.dram_tensor("x", [B, IL, K], F32, kind="ExternalInput").ap()
    v_d = nc.dram_tensor("v", [C, O, B], F32R, kind="ExternalOutput").ap()
    xt_d = nc.dram_tensor("xt", [IL * K, B], BF16).ap()
    cc_in = [nc.dram_tensor(f"cc_in{t}", [16, CB], F32).ap() for t in range(3)]
    cc_out = [nc.dram_tensor(f"cc_out{t}", [16, CB], F32).ap() for t in range(3)]

    with tile.TileContext(nc) as tc:
        with (
            tc.tile_pool(name="const", bufs=1) as constp,
            tc.tile_pool(name="persist", bufs=1) as pers,
            tc.tile_pool(name="small", bufs=4) as smallp,
            tc.tile_pool(name="work", bufs=3) as workp,
            tc.tile_pool(name="fold", bufs=2) as foldp,
            tc.tile_pool(name="soft", bufs=1) as softp,
        ):
            # ---------------- constants
            ident = constp.tile([128, 128], F32)
            masks.make_identity(nc, ident[:])
            identb = constp.tile([128, 128], BF16)
            with nc.allow_low_precision(reason="identity copy"):
                nc.vector.tensor_copy(identb[:], ident[:])
            ones16f = constp.tile([16, 1], F32)
            nc.gpsimd.memset(ones16f[:], 1.0)
            ones16 = constp.tile([16, 1], F32R)
            ones1f = constp.tile([1, 16], F32)
            nc.gpsimd.memset(ones1f[:], 1.0)
            ones1 = constp.tile([1, 16], F32R)
            with nc.allow_low_precision(reason="ones copy"):
                nc.vector.tensor_copy(ones16[:], ones16f[:])
                nc.vector.tensor_copy(ones1[:], ones1f[:])

            # ---------------- persistent state
            xt_i = pers.tile([128, 2 * KB_], BF16)      # [i, (h, k, b)]
            wf = pers.tile([128, 2 * C * 128], BF16)    # [i, (c, h, (k,o))]
            T2 = pers.tile([16, 2 * C * K * 128], BF16)  # [o, (c, h, k, i)]
            w_acc = pers.tile([16, CB], BF16)
            bt = pers.tile([128, 2 * CB], BF16)         # [i, (h, c, b)]
            e_all = pers.tile([128, 2 * CB], BF16)
            ct_all = pers.tile([128, 2 * CB], BF16)

            # ---------------- phase A: x -> xt_d -> xt_i
            phio_cm = contextlib.ExitStack()
            phio = phio_cm.enter_context(tc.tile_pool(name="phio", bufs=3))
            with tc.tile_pool(name="xps", bufs=4, space="PSUM") as xps:
                for bh in range(2):
                    xin = phio.tile([128, IL * K], F32, tag="xin", bufs=2)
                    nc.sync.dma_start(
                        xin[:],
                        x_d[bh * 128 : (bh + 1) * 128].rearrange(
                            "b i k -> b (i k)"
                        ),
                    )
                    xc = phio.tile([128, IL * K // 128 * 128], BF16, tag="xc",
                                   bufs=1)
                    for q in range(16):
                        ps = xps.tile([128, 128], F32)
                        nc.tensor.transpose(
                            ps[:], xin[:, q * 128 : (q + 1) * 128], ident[:]
                        )
                        (nc.scalar.copy if q % 2 == 0 else nc.vector.tensor_copy)(
                            xc[:, q * 128 : (q + 1) * 128], ps[:]
                        )
                    # dst rows (q*128+p), cols [bh*128, bh*128+128)
                    nc.scalar.dma_start(
                        xt_d.rearrange("(q p) b -> p q b", p=128)[
                            :, :, bh * 128 : (bh + 1) * 128
                        ],
                        xc[:].rearrange("p (q b) -> p q b", q=16),
                    )
            for h in range(2):
                # src rows i*8+k with i = h*128+p -> per partition 4KB run
                nc.sync.dma_start(
                    xt_i[:, h * KB_ : (h + 1) * KB_],
                    xt_d[h * 1024 : (h + 1) * 1024].rearrange(
                        "(p k) b -> p (k b)", k=K
                    ),
                )

            # ---------------- phase B: W -> wf (d-reduce trees) -> T, s0
            with (
                tc.tile_pool(name="tps", bufs=2, space="PSUM") as tpsp,
                tc.tile_pool(name="s0ps", bufs=2, space="PSUM") as s0ps,
            ):
                for c in range(C):
                    for h in range(2):
                        t = 2 * c + h
                        wt = phio.tile([128, O * D * K], F32, tag="wt", bufs=2)
                        (nc.sync if h == 0 else nc.scalar).dma_start(
                            wt[:],
                            W_d[c, h * 128 : (h + 1) * 128].rearrange(
                                "p o d k -> p (o d k)"
                            ),
                        )
                        # reduce over d in 4 levels of strided adds
                        eng = nc.vector if t % 3 != 2 else nc.gpsimd
                        v4 = wt[:].rearrange("p (o d k) -> p o d k", o=O, d=D,
                                             k=K)
                        a1 = foldp.tile([128, 1024], F32, tag="a1")
                        a1v = a1[:].rearrange("p (o d k) -> p o d k", o=O, d=8,
                                              k=K)
                        eng.tensor_add(a1v, v4[:, :, 0:8, :], v4[:, :, 8:16, :])
                        a2 = foldp.tile([128, 512], F32, tag="a2")
                        a2v = a2[:].rearrange("p (o d k) -> p o d k", o=O, d=4,
                                              k=K)
                        eng.tensor_add(a2v, a1v[:, :, 0:4, :], a1v[:, :, 4:8, :])
                        a3 = foldp.tile([128, 256], F32, tag="a3")
                        a3v = a3[:].rearrange("p (o d k) -> p o d k", o=O, d=2,
                                              k=K)
                        eng.tensor_add(a3v, a2v[:, :, 0:2, :], a2v[:, :, 2:4, :])
                        # final: f32 -> bf16, output layout (k, o): o str 1, k str 16
                        wfs = wf[:, t * 128 : (t + 1) * 128].rearrange(
                            "p (k u o) -> p o u k", k=K, u=1
                        )
                        with nc.allow_low_precision(reason="wsum bf16"):
                            eng.tensor_add(
                                wfs, a3v[:, :, 0:1, :], a3v[:, :, 1:2, :]
                            )
                        # transpose each k-slice [128,16] -> [16,128] (base 0)
                        tp = tpsp.tile([16, K * 128], BF16, tag="tp")
                        for k in range(K):
                            nc.tensor.transpose(
                                tp[:, k * 128 : (k + 1) * 128],
                                wf[:, t * 128 + k * 16 : t * 128 + (k + 1) * 16],
                                identb[:],
                            )
                        nc.scalar.copy(
                            T2[:, t * K * 128 : (t + 1) * K * 128], tp[:]
                        )
                    # s0: uniform-c iteration 0 partials
                    s0p = s0ps.tile([16, B], F32, tag="s0p")
                    for h in range(2):
                        t = 2 * c + h
                        for k in range(K):
                            nc.tensor.matmul(
                                s0p[:],
                                wf[:, t * 128 + k * 16 : t * 128 + (k + 1) * 16],
                                xt_i[:, h * KB_ + k * B : h * KB_ + (k + 1) * B],
                                start=(h == 0 and k == 0),
                                stop=(h == 1 and k == K - 1),
                            )
                    s0sb = smallp.tile([16, B], F32, tag="s_sb",
                                       name=f"s0sb{c}")
                    nc.scalar.copy(s0sb[:], s0p[:])
                    nc.sync.dma_start(cc_in[0][:, c * B : (c + 1) * B], s0sb[:])

            phio_cm.close()

            # ---------------- allreduce + squash helper (from baseline)
            def allreduce_squash(t, pre, last, sqp):
                nc.gpsimd.collective_compute(
                    "AllReduce",
                    mybir.AluOpType.add,
                    replica_groups=[list(range(NCORES))],
                    ins=[cc_in[t].opt()],
                    outs=[cc_out[t].opt()],
                )
                s_sum = sqp.tile([16, CB], F32, tag="s_sum", name=f"s_sum{t}")
                nc.sync.dma_start(s_sum[:], cc_out[t][:, :])
                sq = sqp.tile([16, CB], F32R, tag="sq", name=f"sq{t}")
                nc.scalar.activation(
                    sq[:], s_sum[:], mybir.ActivationFunctionType.Square,
                    scale=pre,
                )
                with tc.tile_pool(name=f"sqps{t}", bufs=1, space="PSUM") as sqps:
                    ssq_ps = sqps.tile([1, CB], F32, tag="ssq")
                    for j in range(4):
                        nc.tensor.matmul(
                            ssq_ps[:, j * 512 : (j + 1) * 512],
                            ones16[:],
                            sq[:, j * 512 : (j + 1) * 512],
                            start=True, stop=True,
                        )
                    ssq_row = sqp.tile([1, CB], F32R, tag="row_tmp",
                                       name=f"ssq_row{t}")
                    nc.scalar.copy(ssq_row[:], ssq_ps[:])
                ssq = sqp.tile([128, 16], F32R, tag="ssq_rs", name=f"ssq_rs{t}")
                nc.sync.dma_start(
                    ssq[:], ssq_row[:].rearrange("u (p f) -> u p f", p=128)
                )
                den1 = sqp.tile([128, 16], F32, tag="den1", name=f"den1{t}")
                nc.vector.tensor_scalar_add(den1[:], ssq[:], 1.0)
                r1 = sqp.tile([128, 16], F32, tag="r1", name=f"r1{t}")
                nc.vector.reciprocal(r1[:], den1[:])
                rt = sqp.tile([128, 16], F32, tag="rt", name=f"rt{t}")
                nc.scalar.sqrt(rt[:], ssq[:])
                r2 = sqp.tile([128, 16], F32, tag="r2", name=f"r2{t}")
                nc.vector.reciprocal(r2[:], rt[:])
                m1 = sqp.tile([128, 16], F32, tag="m1", name=f"m1{t}")
                nc.vector.tensor_mul(m1[:], ssq[:], r1[:])
                scale_rs = sqp.tile([128, 16], F32R, tag="scale_rs",
                                    name=f"scale_rs{t}")
                nc.vector.tensor_mul(scale_rs[:], m1[:], r2[:])
                if pre != 1.0:
                    nc.vector.tensor_scalar_mul(scale_rs[:], scale_rs[:], pre)
                scale_row = sqp.tile([1, CB], F32R, tag="row_tmp",
                                     name=f"scale_row{t}")
                nc.sync.dma_start(
                    scale_row[:].rearrange("u (p f) -> u p f", p=128),
                    scale_rs[:],
                )
                with tc.tile_pool(name=f"bcps{t}", bufs=1, space="PSUM") as bcps:
                    bc_ps = bcps.tile([16, CB], F32, tag="bc")
                    for j in range(4):
                        nc.tensor.matmul(
                            bc_ps[:, j * 512 : (j + 1) * 512],
                            ones1[:],
                            scale_row[:, j * 512 : (j + 1) * 512],
                            start=True, stop=True,
                        )
                    v_sb = sqp.tile([16, CB], F32R, tag="v_sbr",
                                    name=f"v_sbr{t}")
                    with nc.allow_low_precision(reason="f32r full range"):
                        nc.vector.tensor_mul(v_sb[:], s_sum[:], bc_ps[:])
                    if last:
                        for c in range(C):
                            nc.sync.dma_start(
                                v_d[c], v_sb[:, c * B : (c + 1) * B]
                            )
                    elif t == 0:
                        with nc.allow_low_precision(reason="w bf16"):
                            nc.vector.tensor_copy(w_acc[:], v_sb[:])
                    else:
                        with nc.allow_low_precision(reason="w accum"):
                            nc.vector.tensor_add(w_acc[:], w_acc[:], v_sb[:])

            with tc.tile_pool(name="sq0", bufs=1) as sqp0:
                allreduce_squash(0, 1.0 / C, last=False, sqp=sqp0)

            # ---------------- routing iterations 1 and 2
            for it in range(1, 3):
                # ---- phase 1: bt[i, (h, c, b)] = sum_k xt*G
                with tc.tile_pool(name=f"gps{it}", bufs=4,
                                  space="PSUM") as gps:
                    for c in range(C):
                        for h in range(2):
                            t = 2 * c + h
                            f1 = foldp.tile([128, 512], BF16, tag="f1",
                                            name=f"f1_{it}_{t}")
                            for kh in range(2):
                                g = gps.tile([128, 1024], F32, tag="g")
                                for kk in range(4):
                                    k = kh * 4 + kk
                                    nc.tensor.matmul(
                                        g[:, kk * B : (kk + 1) * B],
                                        T2[:, (t * K + k) * 128 :
                                           (t * K + k + 1) * 128],
                                        w_acc[:, c * B : (c + 1) * B],
                                        start=True, stop=True,
                                    )
                                # Pool can't read PSUM: Act narrows G to bf16
                                # in SBUF, then DVE/Pool multiply all-bf16.
                                g16 = workp.tile([128, 1024], BF16, tag="g16")
                                nc.scalar.copy(g16[:], g[:])
                                p16 = workp.tile([128, 1024], BF16, tag="p16")
                                peng = nc.gpsimd if (2 * t + kh) % 2 == 0 \
                                    else nc.vector
                                with nc.allow_low_precision(reason="P bf16"):
                                    peng.tensor_mul(
                                        p16[:].rearrange("p (k b) -> p k b",
                                                         k=4),
                                        xt_i[:, h * KB_ + kh * 1024 :
                                             h * KB_ + (kh + 1) * 1024]
                                        .rearrange("p (k b) -> p k b", k=4),
                                        g16[:].rearrange("p (k b) -> p k b",
                                                         k=4),
                                    )
                                with nc.allow_low_precision(reason="fold"):
                                    nc.vector.tensor_add(
                                        f1[:, kh * 256 : (kh + 1) * 256]
                                        .rearrange("p (u b) -> p u b", u=1),
                                        p16[:, 0:256].rearrange(
                                            "p (u b) -> p u b", u=1),
                                        p16[:, 256:512].rearrange(
                                            "p (u b) -> p u b", u=1),
                                    )
                                    nc.vector.tensor_add(
                                        f1[:, kh * 256 : (kh + 1) * 256]
                                        .rearrange("p (u b) -> p u b", u=1),
                                        f1[:, kh * 256 : (kh + 1) * 256]
                                        .rearrange("p (u b) -> p u b", u=1),
                                        p16[:, 512:768].rearrange(
                                            "p (u b) -> p u b", u=1),
                                    )
                                    nc.vector.tensor_add(
                                        f1[:, kh * 256 : (kh + 1) * 256]
                                        .rearrange("p (u b) -> p u b", u=1),
                                        f1[:, kh * 256 : (kh + 1) * 256]
                                        .rearrange("p (u b) -> p u b", u=1),
                                        p16[:, 768:1024].rearrange(
                                            "p (u b) -> p u b", u=1),
                                    )
                            with nc.allow_low_precision(reason="bt bf16"):
                                nc.vector.tensor_add(
                                    bt[:, h * CB + c * B :
                                       h * CB + (c + 1) * B],
                                    f1[:, 0:256],
                                    f1[:, 256:512],
                                )

                # ---- phase 2: softmax over classes (tree max-sub, exp, den)
                for h in range(2):
                    bth = bt[:, h * CB : (h + 1) * CB]
                    m1 = softp.tile([128, 4 * B], BF16, tag="m1")
                    with nc.allow_low_precision(reason="softmax max"):
                        nc.vector.tensor_max(
                            m1[:], bth[:, 0 : 4 * B], bth[:, 4 * B : 8 * B]
                        )
                        m2 = softp.tile([128, 2 * B], BF16, tag="m2")
                        nc.vector.tensor_max(
                            m2[:], m1[:, 0 : 2 * B], m1[:, 2 * B : 4 * B]
                        )
                        rmax = softp.tile([128, B], BF16, tag="rmax")
                        nc.vector.tensor_max(
                            rmax[:], m2[:, 0:B], m2[:, B : 2 * B]
                        )
                        sub = softp.tile([128, CB], BF16, tag="sub")
                        nc.vector.tensor_sub(
                            sub[:].rearrange("p (c b) -> p c b", c=C),
                            bth.rearrange("p (c b) -> p c b", c=C),
                            rmax[:].rearrange("p (u b) -> p u b", u=1)
                            .broadcast_to([128, C, B]),
                        )
                    eh = e_all[:, h * CB : (h + 1) * CB]
                    nc.scalar.activation(
                        eh, sub[:], mybir.ActivationFunctionType.Exp
                    )
                    d1 = softp.tile([128, 4 * B], F32, tag="d1")
                    nc.vector.tensor_add(
                        d1[:], eh[:, 0 : 4 * B], eh[:, 4 * B : 8 * B]
                    )
                    d2 = softp.tile([128, 2 * B], F32, tag="d2")
                    nc.vector.tensor_add(
                        d2[:], d1[:, 0 : 2 * B], d1[:, 2 * B : 4 * B]
                    )
                    den = softp.tile([128, B], F32, tag="den")
                    nc.vector.tensor_add(den[:], d2[:, 0:B], d2[:, B : 2 * B])
                    rec = softp.tile([128, B], F32, tag="rec")
                    nc.vector.reciprocal(rec[:], den[:])
                    recb = softp.tile([128, B], BF16, tag="recb")
                    with nc.allow_low_precision(reason="rec bf16"):
                        nc.vector.tensor_copy(recb[:], rec[:])
                        nc.vector.tensor_mul(
                            ct_all[:, h * CB : (h + 1) * CB].rearrange(
                                "p (c b) -> p c b", c=C
                            ),
                            e_all[:, h * CB : (h + 1) * CB].rearrange(
                                "p (c b) -> p c b", c=C
                            ),
                            recb[:].rearrange("p (u b) -> p u b", u=1)
                            .broadcast_to([128, C, B]),
                        )

                # ---- phase 3: y = ct*x, s_c = sum_{h,k} wf^T y
                with tc.tile_pool(name=f"sps{it}", bufs=2,
                                  space="PSUM") as sps:
                    for c in range(C):
                        s_ps = sps.tile([16, B], F32, tag="s",
                                        name=f"s_ps{it}_{c}")
                        for h in range(2):
                            t = 2 * c + h
                            y = workp.tile([128, KB_], BF16, tag="y")
                            yeng = nc.gpsimd if (t % 4) == 3 else nc.vector
                            with nc.allow_low_precision(reason="y bf16"):
                                yeng.tensor_mul(
                                    y[:].rearrange("p (k b) -> p k b", k=K),
                                    xt_i[:, h * KB_ : (h + 1) * KB_]
                                    .rearrange("p (k b) -> p k b", k=K),
                                    ct_all[:, h * CB + c * B :
                                           h * CB + (c + 1) * B]
                                    .rearrange("p (u b) -> p u b", u=1)
                                    .broadcast_to([128, K, B]),
                                )
                            for k in range(K):
                                nc.tensor.matmul(
                                    s_ps[:],
                                    wf[:, t * 128 + k * 16 :
                                       t * 128 + (k + 1) * 16],
                                    y[:, k * B : (k + 1) * B],
                                    start=(h == 0 and k == 0),
                                    stop=(h == 1 and k == K - 1),
                                )
                        s_sb = smallp.tile([16, B], F32, tag="s_sb",
                                           name=f"s_sb{it}_{c}")
                        nc.scalar.copy(s_sb[:], s_ps[:])
                        nc.sync.dma_start(
                            cc_in[it][:, c * B : (c + 1) * B], s_sb[:]
                        )
                with tc.tile_pool(name=f"sq{it}", bufs=1) as sqpi:
                    allreduce_squash(it, 1.0, last=(it == 2), sqp=sqpi)
    return fixup_multi_waits(nc) if fixup else nc


_NC = None


def kernel(x: np.ndarray, W: np.ndarray, _timings=None) -> np.ndarray:
    global _NC
    x = np.asarray(x, np.float32)
    W = np.asarray(W, np.float32)
    if _NC is None:
        _NC = build_all()
    in_maps = []
    for j in range(NCORES):
        sl = slice(j * IL, (j + 1) * IL)
        in_maps.append(
            {
                "W": np.ascontiguousarray(W[:, 0, sl]),
                "x": np.ascontiguousarray(x[:, sl, :]),
            }
        )
    res = run_bass_kernel_spmd(
        _NC, in_maps, core_ids=list(range(NCORES)),
        trace=_timings is not None,
    )
    if _timings is not None:
        _timings.append(res.exec_time_ns)
    v = res.results[0]["v"].astype(np.float32)  # [C, O, B]
    return np.ascontiguousarray(v.transpose(2, 0, 1))


# revision 36
# speedup vs baseline: 1.2308x; 1.2226x over previous
"""CapsuleLayer (dynamic routing) on 8 trn2 NeuronCores.

Math: u_hat[b,c,i,o] = sum_{d,k} W[c,0,i,o,d,k] x[b,i,k]
             = sum_k Wsum[c,i,o,k] x[b,i,k],  Wsum = W.sum(d)   (134MB -> 8.4MB)
Routing logits are cumulative: b_t = u_hat . (sum_{tau<t} v_tau), so each
iteration only needs the running vector-sum w.  Everything is sharded over
IN_CAPS (i) across 8 cores; only s[b,c,o] (131KB) crosses cores via AllReduce.

Per-core layouts (partition dim = i throughout the routing iterations):
  xt_i[h]   [128(i), (k,b)]  bf16  - x transposed via PE + 1MB HBM round-trip
  wf        [128(i), (c,h,(k,o))] bf16 - Wsum, d-reduced on DVE/Pool trees
  T_all     [128(k,o), (c,h,i)]  bf16 - PE-transpose of wf (G stationaries)
  w_acc     [16(o), (c,b)]  bf16 - running sum of squash outputs v
Iteration t:
  G_k[i,b]  = T[c,h,k-slice]^T w_acc[c]          (PE, K=o=16)
  P         = xt_i * G (PSUM f32 read, DVE/Pool), bt = sum_k P (tree adds)
  c_t       = softmax_c(bt)  (exp on Act, den tree, no max-subtraction)
  y_c       = ct_c (bcast over k) * xt_i         (DVE, all bf16)
  s_c[o,b] += wf[c,h,k-slice]^T y_c              (PE, accumulate 16 matmuls)
AllReduce s (131KB f32) -> squash -> w_acc (or v output on last iter).
"""

import contextlib
import sys
import types

import numpy as np
import ml_dtypes  # noqa: F401  (bf16 array dtype for I/O maps)


def _install_ntff_shim():
    """The image's antenv lacks axon_hooks; provide a minimal equivalent so
    run_bass_kernel_spmd(trace=True) can capture NTFF profiles via the
    injected libaxon_pjrt.so.  No-op if the real module exists or the .so
    is unavailable (grading path uses trace=False and never hits this)."""
    try:
        import antenv.axon_hooks  # noqa: F401

        return
    except Exception:
        pass
    import ctypes

    mod = types.ModuleType("antenv.axon_hooks")
    holder = [None, False]

    def set_axon_ntff_profile_hook(h):
        holder[0], holder[1] = h, True

    def _make_hook():
        try:
            lib = ctypes.CDLL("/opt/axon/libaxon_pjrt.so")
        except OSError:
            return None
        if not hasattr(lib, "axon_start_nrt_profile"):
            return None
        lib.axon_start_nrt_profile.argtypes = [
            ctypes.POINTER(ctypes.c_int64),
            ctypes.c_size_t,
        ]
        lib.axon_start_nrt_profile.restype = ctypes.c_int64
        lib.axon_stop_nrt_profile.argtypes = [ctypes.c_char_p]
        lib.axon_stop_nrt_profile.restype = ctypes.c_int64

        @contextlib.contextmanager
        def _hook(output_dir, device_ids):
            import jax

            jax.devices()
            if device_ids:
                ids = (ctypes.c_int64 * len(device_ids))(*device_ids)
                rc = lib.axon_start_nrt_profile(ids, len(device_ids))
            else:
                rc = lib.axon_start_nrt_profile(None, 0)
            if rc != 0:
                raise RuntimeError(f"axon_start_nrt_profile rc={rc}")
            try:
                yield
            finally:
                n = lib.axon_stop_nrt_profile(str(output_dir).encode())
                print(
                    f"profile: {n} file(s) written to {output_dir}",
                    file=sys.stderr,
                )

        return _hook

    def get_axon_ntff_profile_hook():
        if not holder[1]:
            holder[0], holder[1] = _make_hook(), True
        return holder[0]

    mod.set_axon_ntff_profile_hook = set_axon_ntff_profile_hook
    mod.get_axon_ntff_profile_hook = get_axon_ntff_profile_hook
    sys.modules["antenv.axon_hooks"] = mod


try:
    _install_ntff_shim()
except Exception:
    pass

import concourse.bass as bass
import concourse.mybir as mybir
import concourse.tile as tile
from concourse import masks
from concourse.bass_utils import run_bass_kernel_spmd
from bass_rust import ScopedClock

# ---------------------------------------------------------------- constants
C, I, O, D, K, B = 8, 2048, 16, 16, 8, 256
NCORES = 8
IL = I // NCORES          # 256 i's per core
F32 = mybir.dt.float32
F32R = mybir.dt.float32r
BF16 = mybir.dt.bfloat16
CB = C * B
KB_ = K * B               # 2048

# ------------------------------------------------- tile tail-drain workaround
_MAX_WAITS = 1


def _patched_drain_and_barrier(self, tick_clock, wait_clock):
    nc = self.nc
    drain_inst = nc.sync.drain()
    wait_clock.add_sem_waits(
        drain_inst.ins, ScopedClock({None: tick_clock.global_clock})
    )
    si = drain_inst.ins.sync_info
    if si is not None and si.on_wait and len(si.on_wait) > _MAX_WAITS:
        waits = list(si.on_wait)
        si.on_wait = waits[:_MAX_WAITS]
        for i in range(_MAX_WAITS, len(waits), _MAX_WAITS):
            extra = nc.sync.drain()
            extra.ins.sync_info = mybir.SyncInfo(
                on_wait=waits[i : i + _MAX_WAITS], on_update=[]
            )
    nc.all_engine_barrier()
    assert self.sems is not None
    popped = nc._tile_sem_poison_stack.pop()
    assert popped is self._sem_poison
    nc.clear_and_free_semaphores(list(self.sems.allocated().values()))
    nc.all_engine_barrier()


tile.TileContext._drain_and_barrier = _patched_drain_and_barrier

_fix_ctr = [0]


def fixup_multi_waits(nc):
    """walrus in this toolchain accepts at most one sem wait per instruction;
    hoist extra waits onto same-engine drains placed just before."""
    for f in nc.m.functions:
        for bb in f.blocks:
            out = []
            for inst in bb.instructions:
                si = inst.sync_info
                if si is not None and si.on_wait and len(si.on_wait) > _MAX_WAITS:
                    waits = list(si.on_wait)
                    for i in range(0, len(waits) - _MAX_WAITS, _MAX_WAITS):
                        _fix_ctr[0] += 1
                        d = mybir.InstDrain(
                            name=f"waitsplit_{_fix_ctr[0]}", ins=[], outs=[]
                        )
                        d.engine = inst.engine
                        d.sync_info = mybir.SyncInfo(
                            on_wait=waits[i : i + _MAX_WAITS], on_update=[]
                        )
                        out.append(d)
                    si.on_wait = waits[len(waits) - _MAX_WAITS :]
                out.append(inst)
            bb.instructions[:] = out
    return nc


def build_all(fixup=True):
    nc = bass.Bass("TRN2", target_bir_lowering=False, debug=False,
                   num_devices=NCORES)
    W_d = nc.dram_tensor("W", [C, IL, O, D, K], F32, kind="ExternalInput").ap()
    x_d = nc.dram_tensor("x", [B, IL, K], F32, kind="ExternalInput").ap()
    v_d = nc.dram_tensor("v", [C, O, B], F32R, kind="ExternalOutput").ap()
    xt_d = nc.dram_tensor("xt", [IL * K, B], BF16).ap()
    cc_in = [nc.dram_tensor(f"cc_in{t}", [16, CB], F32).ap() for t in range(3)]
    cc_out = [nc.dram_tensor(f"cc_out{t}", [16, CB], F32).ap() for t in range(3)]

    with tile.TileContext(nc) as tc:
        with (
            tc.tile_pool(name="const", bufs=1) as constp,
            tc.tile_pool(name="persist", bufs=1) as pers,
            tc.tile_pool(name="small", bufs=4) as smallp,
        ):
            # ---------------- constants
            ident = constp.tile([128, 128], F32)
            masks.make_identity(nc, ident[:])
            identb = constp.tile([128, 128], BF16)
            with nc.allow_low_precision(reason="identity copy"):
                nc.vector.tensor_copy(identb[:], ident[:])
            ones16f = constp.tile([16, 1], F32)
            nc.gpsimd.memset(ones16f[:], 1.0)
            ones16 = constp.tile([16, 1], F32R)
            ones1f = constp.tile([1, 16], F32)
            nc.gpsimd.memset(ones1f[:], 1.0)
            ones1 = constp.tile([1, 16], F32R)
            with nc.allow_low_precision(reason="ones copy"):
                nc.vector.tensor_copy(ones16[:], ones16f[:])
                nc.vector.tensor_copy(ones1[:], ones1f[:])

            # ---------------- persistent state
            xt_i = pers.tile([128, 2 * KB_], BF16)      # [i, (h, k, b)]
            wf = pers.tile([128, 2 * C * 128], BF16)    # [i, (c, h, (k,o))]
            T2 = pers.tile([16, 2 * C * K * 128], BF16)  # [o, (c, h, k, i)]
            w_acc = pers.tile([16, CB], BF16)
            bt = pers.tile([128, 2 * CB], BF16)         # [i, (h, c, b)]
            e_all = pers.tile([128, 2 * CB], BF16)      # exp, then ct in-place

            # ---------------- phase A: x -> xt_d -> xt_i
            phio_cm = contextlib.ExitStack()
            phio = phio_cm.enter_context(tc.tile_pool(name="phio", bufs=3))
            wtree = phio_cm.enter_context(tc.tile_pool(name="wtree", bufs=2))
            with tc.tile_pool(name="xps", bufs=4, space="PSUM") as xps:
                for bh in range(2):
                    xin = phio.tile([128, IL * K], F32, tag="xin", bufs=2)
                    nc.sync.dma_start(
                        xin[:],
                        x_d[bh * 128 : (bh + 1) * 128].rearrange(
                            "b i k -> b (i k)"
                        ),
                    )
                    xc = phio.tile([128, IL * K // 128 * 128], BF16, tag="xc",
                                   bufs=1)
                    for q in range(16):
                        ps = xps.tile([128, 128], F32)
                        nc.tensor.transpose(
                            ps[:], xin[:, q * 128 : (q + 1) * 128], ident[:]
                        )
                        nc.scalar.copy(xc[:, q * 128 : (q + 1) * 128], ps[:])
                    # dst rows (q*128+p), cols [bh*128, bh*128+128)
                    nc.scalar.dma_start(
                        xt_d.rearrange("(q p) b -> p q b", p=128)[
                            :, :, bh * 128 : (bh + 1) * 128
                        ],
                        xc[:].rearrange("p (q b) -> p q b", q=16),
                    )
            for h in range(2):
                # src rows i*8+k with i = h*128+p -> per partition 4KB run
                nc.sync.dma_start(
                    xt_i[:, h * KB_ : (h + 1) * KB_],
                    xt_d[h * 1024 : (h + 1) * 1024].rearrange(
                        "(p k) b -> p (k b)", k=K
                    ),
                )

            # ---------------- phase B: W -> wf (d-reduce trees) -> T, s0
            with (
                tc.tile_pool(name="tps", bufs=2, space="PSUM") as tpsp,
                tc.tile_pool(name="s0ps", bufs=2, space="PSUM") as s0ps,
            ):
                for c in range(C):
                    for h in range(2):
                        t = 2 * c + h
                        wt = phio.tile([128, O * D * K], F32, tag="wt", bufs=2)
                        (nc.sync if h == 0 else nc.scalar).dma_start(
                            wt[:],
                            W_d[c, h * 128 : (h + 1) * 128].rearrange(
                                "p o d k -> p (o d k)"
                            ),
                        )
                        # reduce over d in 4 levels of strided adds.  All on
                        # DVE: gpsimd must stay empty before the first
                        # collective trigger or the cross-core barrier (and
                        # with it AllReduce 0) queues behind setup work.
                        eng = nc.vector
                        v4 = wt[:].rearrange("p (o d k) -> p o d k", o=O, d=D,
                                             k=K)
                        a1 = wtree.tile([128, 1024], F32, tag="a1")
                        a1v = a1[:].rearrange("p (o d k) -> p o d k", o=O, d=8,
                                              k=K)
                        eng.tensor_add(a1v, v4[:, :, 0:8, :], v4[:, :, 8:16, :])
                        a2 = wtree.tile([128, 512], F32, tag="a2")
                        a2v = a2[:].rearrange("p (o d k) -> p o d k", o=O, d=4,
                                              k=K)
                        eng.tensor_add(a2v, a1v[:, :, 0:4, :], a1v[:, :, 4:8, :])
                        a3 = wtree.tile([128, 256], F32, tag="a3")
                        a3v = a3[:].rearrange("p (o d k) -> p o d k", o=O, d=2,
                                              k=K)
                        eng.tensor_add(a3v, a2v[:, :, 0:2, :], a2v[:, :, 2:4, :])
                        # final: f32 -> bf16, output layout (k, o): o str 1, k str 16
                        wfs = wf[:, t * 128 : (t + 1) * 128].rearrange(
                            "p (k u o) -> p o u k", k=K, u=1
                        )
                        with nc.allow_low_precision(reason="wsum bf16"):
                            eng.tensor_add(
                                wfs, a3v[:, :, 0:1, :], a3v[:, :, 1:2, :]
                            )
                        # transpose each k-slice [128,16] -> [16,128] (base 0)
                        tp = tpsp.tile([16, K * 128], BF16, tag="tp")
                        for k in range(K):
                            nc.tensor.transpose(
                                tp[:, k * 128 : (k + 1) * 128],
                                wf[:, t * 128 + k * 16 : t * 128 + (k + 1) * 16],
                                identb[:],
                            )
                        nc.scalar.copy(
                            T2[:, t * K * 128 : (t + 1) * K * 128], tp[:]
                        )
                    # s0: uniform-c iteration 0 partials
                    s0p = s0ps.tile([16, B], F32, tag="s0p")
                    for h in range(2):
                        t = 2 * c + h
                        for k in range(K):
                            nc.tensor.matmul(
                                s0p[:],
                                wf[:, t * 128 + k * 16 : t * 128 + (k + 1) * 16],
                                xt_i[:, h * KB_ + k * B : h * KB_ + (k + 1) * B],
                                start=(h == 0 and k == 0),
                                stop=(h == 1 and k == K - 1),
                            )
                    s0sb = smallp.tile([16, B], F32, tag="s_sb",
                                       name=f"s0sb{c}")
                    nc.scalar.copy(s0sb[:], s0p[:])
                    nc.sync.dma_start(cc_in[0][:, c * B : (c + 1) * B], s0sb[:])

            phio_cm.close()

            # ---------------- allreduce + squash helper (from baseline)
            def allreduce_squash(t, pre, last, sqp):
                nc.gpsimd.collective_compute(
                    "AllReduce",
                    mybir.AluOpType.add,
                    replica_groups=[list(range(NCORES))],
                    ins=[cc_in[t].opt()],
                    outs=[cc_out[t].opt()],
                )
                s_sum = sqp.tile([16, CB], F32, tag="s_sum", name=f"s_sum{t}")
                nc.sync.dma_start(s_sum[:], cc_out[t][:, :])
                sq = sqp.tile([16, CB], F32R, tag="sq", name=f"sq{t}")
                nc.scalar.activation(
                    sq[:], s_sum[:], mybir.ActivationFunctionType.Square,
                    scale=pre,
                )
                with tc.tile_pool(name=f"sqps{t}", bufs=1, space="PSUM") as sqps:
                    ssq_ps = sqps.tile([1, CB], F32, tag="ssq")
                    for j in range(4):
                        nc.tensor.matmul(
                            ssq_ps[:, j * 512 : (j + 1) * 512],
                            ones16[:],
                            sq[:, j * 512 : (j + 1) * 512],
                            start=True, stop=True,
                        )
                    ssq_row = sqp.tile([1, CB], F32R, tag="row_tmp",
                                       name=f"ssq_row{t}")
                    nc.scalar.copy(ssq_row[:], ssq_ps[:])
                ssq = sqp.tile([128, 16], F32R, tag="ssq_rs", name=f"ssq_rs{t}")
                nc.sync.dma_start(
                    ssq[:], ssq_row[:].rearrange("u (p f) -> u p f", p=128)
                )
                den1 = sqp.tile([128, 16], F32, tag="den1", name=f"den1{t}")
                nc.vector.tensor_scalar_add(den1[:], ssq[:], 1.0)
                r1 = sqp.tile([128, 16], F32, tag="r1", name=f"r1{t}")
                nc.vector.reciprocal(r1[:], den1[:])
                rt = sqp.tile([128, 16], F32, tag="rt", name=f"rt{t}")
                nc.scalar.sqrt(rt[:], ssq[:])
                r2 = sqp.tile([128, 16], F32, tag="r2", name=f"r2{t}")
                nc.vector.reciprocal(r2[:], rt[:])
                m1 = sqp.tile([128, 16], F32, tag="m1", name=f"m1{t}")
                nc.vector.tensor_mul(m1[:], ssq[:], r1[:])
                scale_rs = sqp.tile([128, 16], F32R, tag="scale_rs",
                                    name=f"scale_rs{t}")
                nc.vector.tensor_mul(scale_rs[:], m1[:], r2[:])
                if pre != 1.0:
                    nc.vector.tensor_scalar_mul(scale_rs[:], scale_rs[:], pre)
                scale_row = sqp.tile([1, CB], F32R, tag="row_tmp",
                                     name=f"scale_row{t}")
                nc.sync.dma_start(
                    scale_row[:].rearrange("u (p f) -> u p f", p=128),
                    scale_rs[:],
                )
                with tc.tile_pool(name=f"bcps{t}", bufs=1, space="PSUM") as bcps:
                    bc_ps = bcps.tile([16, CB], F32, tag="bc")
                    for j in range(4):
                        nc.tensor.matmul(
                            bc_ps[:, j * 512 : (j + 1) * 512],
                            ones1[:],
                            scale_row[:, j * 512 : (j + 1) * 512],
                            start=True, stop=True,
                        )
                    v_sb = sqp.tile([16, CB], F32R, tag="v_sbr",
                                    name=f"v_sbr{t}")
                    with nc.allow_low_precision(reason="f32r full range"):
                        nc.vector.tensor_mul(v_sb[:], s_sum[:], bc_ps[:])
                    if last:
                        for c in range(C):
                            nc.sync.dma_start(
                                v_d[c], v_sb[:, c * B : (c + 1) * B]
                            )
                    elif t == 0:
                        with nc.allow_low_precision(reason="w bf16"):
                            nc.vector.tensor_copy(w_acc[:], v_sb[:])
                    else:
                        with nc.allow_low_precision(reason="w accum"):
                            nc.vector.tensor_add(w_acc[:], w_acc[:], v_sb[:])

            with tc.tile_pool(name="sq0", bufs=1) as sqp0:
                allreduce_squash(0, 1.0 / C, last=False, sqp=sqp0)

            # ---------------- routing iterations 1 and 2
            # DVE per-instruction overhead is ~0.4us, so everything below
            # works on the largest slices SBUF allows.
            with (
                tc.tile_pool(name="workp", bufs=1) as workp,
                tc.tile_pool(name="p2p", bufs=2) as p2p,
                tc.tile_pool(name="fldp", bufs=1) as fldp,
                tc.tile_pool(name="softp", bufs=1) as softp,
            ):
                for it in range(1, 3):
                    # ---- phase 1: bt[i, (h, c, b)] = sum_k xt*G
                    with tc.tile_pool(name=f"gps{it}", bufs=2,
                                      space="PSUM") as gps:
                        for h in range(2):
                            for cg in range(4):  # c-pairs
                                p2 = p2p.tile([128, 2 * KB_], BF16, tag="p2")
                                for cc in range(2):
                                    c = cg * 2 + cc
                                    t = 2 * c + h
                                    g = gps.tile([128, KB_], F32, tag="g")
                                    for k in range(K):
                                        nc.tensor.matmul(
                                            g[:, k * B : (k + 1) * B],
                                            T2[:, (t * K + k) * 128 :
                                               (t * K + k + 1) * 128],
                                            w_acc[:, c * B : (c + 1) * B],
                                            start=True, stop=True,
                                        )
                                    # Pool can't read PSUM: Act narrows G to
                                    # bf16 SBUF, then DVE/Pool go all-bf16.
                                    g16 = workp.tile([128, KB_], BF16,
                                                     tag="g16", bufs=3)
                                    nc.scalar.copy(g16[:], g[:])
                                    peng = (nc.gpsimd if t % 4 == 1
                                            else nc.vector)
                                    with nc.allow_low_precision(reason="P"):
                                        peng.tensor_mul(
                                            p2[:, cc * KB_ : (cc + 1) * KB_],
                                            xt_i[:, h * KB_ : (h + 1) * KB_],
                                            g16[:],
                                        )
                                # fold over k: 3 big strided adds per c-pair
                                p2v = p2[:].rearrange(
                                    "p (c k b) -> p c k b", c=2, k=K
                                )
                                fl1 = fldp.tile([128, KB_], BF16, tag="fl1")
                                f1v = fl1[:].rearrange(
                                    "p (c k b) -> p c k b", c=2, k=4
                                )
                                fl2 = fldp.tile([128, KB_ // 2], BF16,
                                                tag="fl2")
                                f2v = fl2[:].rearrange(
                                    "p (c k b) -> p c k b", c=2, k=2
                                )
                                btv = bt[:, h * CB + cg * 2 * B :
                                         h * CB + (cg * 2 + 2) * B] \
                                    .rearrange("p (c u b) -> p c u b",
                                               c=2, u=1)
                                with nc.allow_low_precision(reason="fold"):
                                    nc.vector.tensor_add(
                                        f1v, p2v[:, :, 0:4, :],
                                        p2v[:, :, 4:8, :]
                                    )
                                    nc.vector.tensor_add(
                                        f2v, f1v[:, :, 0:2, :],
                                        f1v[:, :, 2:4, :]
                                    )
                                    nc.vector.tensor_add(
                                        btv, f2v[:, :, 0:1, :],
                                        f2v[:, :, 1:2, :]
                                    )

                    # ---- phase 2: softmax over classes, both halves at once
                    btv = bt[:].rearrange("p (h c b) -> p h c b", h=2, c=C)
                    m1 = softp.tile([128, 2 * 4 * B], BF16, tag="m1")
                    m1v = m1[:].rearrange("p (h c b) -> p h c b", h=2, c=4)
                    m2 = softp.tile([128, 2 * 2 * B], BF16, tag="m2")
                    m2v = m2[:].rearrange("p (h c b) -> p h c b", h=2, c=2)
                    rmax = softp.tile([128, 2 * B], BF16, tag="rmax")
                    rmv = rmax[:].rearrange("p (h u b) -> p h u b", h=2, u=1)
                    sub = softp.tile([128, 2 * CB], BF16, tag="sub")
                    subv = sub[:].rearrange("p (h c b) -> p h c b", h=2, c=C)
                    with nc.allow_low_precision(reason="softmax max"):
                        nc.vector.tensor_max(
                            m1v, btv[:, :, 0:4, :], btv[:, :, 4:8, :]
                        )
                        nc.vector.tensor_max(
                            m2v, m1v[:, :, 0:2, :], m1v[:, :, 2:4, :]
                        )
                        nc.vector.tensor_max(
                            rmv, m2v[:, :, 0:1, :], m2v[:, :, 1:2, :]
                        )
                        nc.vector.tensor_sub(
                            subv, btv,
                            rmax[:].rearrange("p (h b) -> p h b", h=2)
                            .unsqueeze(2).broadcast_to([128, 2, C, B]),
                        )
                    nc.scalar.activation(
                        e_all[:], sub[:], mybir.ActivationFunctionType.Exp
                    )
                    ev = e_all[:].rearrange("p (h c b) -> p h c b", h=2, c=C)
                    d1 = softp.tile([128, 2 * 4 * B], BF16, tag="m1",
                                    name=f"d1_{it}")
                    d1v = d1[:].rearrange("p (h c b) -> p h c b", h=2, c=4)
                    d2 = softp.tile([128, 2 * 2 * B], BF16, tag="m2",
                                    name=f"d2_{it}")
                    d2v = d2[:].rearrange("p (h c b) -> p h c b", h=2, c=2)
                    den = softp.tile([128, 2 * B], F32, tag="den")
                    denv = den[:].rearrange("p (h u b) -> p h u b", h=2, u=1)
                    with nc.allow_low_precision(reason="den partials bf16"):
                        nc.vector.tensor_add(
                            d1v, ev[:, :, 0:4, :], ev[:, :, 4:8, :]
                        )
                        nc.vector.tensor_add(
                            d2v, d1v[:, :, 0:2, :], d1v[:, :, 2:4, :]
                        )
                    nc.vector.tensor_add(
                        denv, d2v[:, :, 0:1, :], d2v[:, :, 1:2, :]
                    )
                    rec = softp.tile([128, 2 * B], F32, tag="rec")
                    nc.vector.reciprocal(rec[:], den[:])
                    recb = softp.tile([128, 2 * B], BF16, tag="recb")
                    with nc.allow_low_precision(reason="rec bf16"):
                        nc.vector.tensor_copy(recb[:], rec[:])
                        # ct overwrites e in place
                        nc.vector.tensor_mul(
                            ev, ev,
                            recb[:].rearrange("p (h b) -> p h b", h=2)
                            .unsqueeze(2).broadcast_to([128, 2, C, B]),
                        )

                    # ---- phase 3: y = ct*x, s_c = sum_{h,k} wf^T y
                    with tc.tile_pool(name=f"sps{it}", bufs=1,
                                      space="PSUM") as sps:
                        s_ps = sps.tile([16, CB], F32, tag="s")
                        for c in range(C):
                            y = workp.tile([128, 2 * KB_], BF16, tag="y",
                                           bufs=2)
                            yeng = nc.gpsimd if (c % 4) == 3 else nc.vector
                            with nc.allow_low_precision(reason="y bf16"):
                                yeng.tensor_mul(
                                    y[:].rearrange("p (h k b) -> p h k b",
                                                   h=2, k=K),
                                    xt_i[:].rearrange("p (h k b) -> p h k b",
                                                      h=2, k=K),
                                    e_all[:].rearrange(
                                        "p (h c b) -> p h c b", h=2, c=C
                                    )[:, :, c : c + 1, :]
                                    .broadcast_to([128, 2, K, B]),
                                )
                            for h in range(2):
                                t = 2 * c + h
                                for k in range(K):
                                    nc.tensor.matmul(
                                        s_ps[:, c * B : (c + 1) * B],
                                        wf[:, t * 128 + k * 16 :
                                           t * 128 + (k + 1) * 16],
                                        y[:, h * KB_ + k * B :
                                          h * KB_ + (k + 1) * B],
                                        start=(h == 0 and k == 0),
                                        stop=(h == 1 and k == K - 1),
                                    )
                        s_sb = smallp.tile([16, CB], F32, tag="s_sbf",
                                           name=f"s_sb{it}", bufs=1)
                        nc.scalar.copy(s_sb[:], s_ps[:])
                        nc.sync.dma_start(cc_in[it][:, :], s_sb[:])
                    with tc.tile_pool(name=f"sq{it}", bufs=1) as sqpi:
                        allreduce_squash(it, 1.0, last=(it == 2), sqp=sqpi)
    return fixup_multi_waits(nc) if fixup else nc


_NC = None


def kernel(x: np.ndarray, W: np.ndarray, _timings=None) -> np.ndarray:
    global _NC
    x = np.asarray(x, np.float32)
    W = np.asarray(W, np.float32)
    if _NC is None:
        _NC = build_all()
    in_maps = []
    for j in range(NCORES):
        sl = slice(j * IL, (j + 1) * IL)
        in_maps.append(
            {
                "W": np.ascontiguousarray(W[:, 0, sl]),
                "x": np.ascontiguousarray(x[:, sl, :]),
            }
        )
    res = run_bass_kernel_spmd(
        _NC, in_maps, core_ids=list(range(NCORES)),
        trace=_timings is not None,
    )
    if _timings is not None:
        _timings.append(res.exec_time_ns)
    v = res.results[0]["v"].astype(np.float32)  # [C, O, B]
    return np.ascontiguousarray(v.transpose(2, 0, 1))


# revision 41
# speedup vs baseline: 1.3263x; 1.0776x over previous
"""CapsuleLayer (dynamic routing) on 8 trn2 NeuronCores.

Math: u_hat[b,c,i,o] = sum_{d,k} W[c,0,i,o,d,k] x[b,i,k]
             = sum_k Wsum[c,i,o,k] x[b,i,k],  Wsum = W.sum(d)   (134MB -> 8.4MB)
Routing logits are cumulative: b_t = u_hat . (sum_{tau<t} v_tau), so each
iteration only needs the running vector-sum w.  Everything is sharded over
IN_CAPS (i) across 8 cores; only s[b,c,o] (131KB) crosses cores via AllReduce.

Per-core layouts (partition dim = i throughout the routing iterations):
  xt_i[h]   [128(i), (k,b)]  bf16  - x transposed via PE + 1MB HBM round-trip
  wf        [128(i), (c,h,(k,o))] bf16 - Wsum, d-reduced on DVE/Pool trees
  T_all     [128(k,o), (c,h,i)]  bf16 - PE-transpose of wf (G stationaries)
  w_acc     [16(o), (c,b)]  bf16 - running sum of squash outputs v
Iteration t:
  G_k[i,b]  = T[c,h,k-slice]^T w_acc[c]          (PE, K=o=16)
  P         = xt_i * G (PSUM f32 read, DVE/Pool), bt = sum_k P (tree adds)
  c_t       = softmax_c(bt)  (exp on Act, den tree, no max-subtraction)
  y_c       = ct_c (bcast over k) * xt_i         (DVE, all bf16)
  s_c[o,b] += wf[c,h,k-slice]^T y_c              (PE, accumulate 16 matmuls)
AllReduce s (131KB f32) -> squash -> w_acc (or v output on last iter).
"""

import contextlib
import sys
import types

import numpy as np
import ml_dtypes  # noqa: F401  (bf16 array dtype for I/O maps)


def _install_ntff_shim():
    """The image's antenv lacks axon_hooks; provide a minimal equivalent so
    run_bass_kernel_spmd(trace=True) can capture NTFF profiles via the
    injected libaxon_pjrt.so.  No-op if the real module exists or the .so
    is unavailable (grading path uses trace=False and never hits this)."""
    try:
        import antenv.axon_hooks  # noqa: F401

        return
    except Exception:
        pass
    import ctypes

    mod = types.ModuleType("antenv.axon_hooks")
    holder = [None, False]

    def set_axon_ntff_profile_hook(h):
        holder[0], holder[1] = h, True

    def _make_hook():
        try:
            lib = ctypes.CDLL("/opt/axon/libaxon_pjrt.so")
        except OSError:
            return None
        if not hasattr(lib, "axon_start_nrt_profile"):
            return None
        lib.axon_start_nrt_profile.argtypes = [
            ctypes.POINTER(ctypes.c_int64),
            ctypes.c_size_t,
        ]
        lib.axon_start_nrt_profile.restype = ctypes.c_int64
        lib.axon_stop_nrt_profile.argtypes = [ctypes.c_char_p]
        lib.axon_stop_nrt_profile.restype = ctypes.c_int64

        @contextlib.contextmanager
        def _hook(output_dir, device_ids):
            import jax

            jax.devices()
            if device_ids:
                ids = (ctypes.c_int64 * len(device_ids))(*device_ids)
                rc = lib.axon_start_nrt_profile(ids, len(device_ids))
            else:
                rc = lib.axon_start_nrt_profile(None, 0)
            if rc != 0:
                raise RuntimeError(f"axon_start_nrt_profile rc={rc}")
            try:
                yield
            finally:
                n = lib.axon_stop_nrt_profile(str(output_dir).encode())
                print(
                    f"profile: {n} file(s) written to {output_dir}",
                    file=sys.stderr,
                )

        return _hook

    def get_axon_ntff_profile_hook():
        if not holder[1]:
            holder[0], holder[1] = _make_hook(), True
        return holder[0]

    mod.set_axon_ntff_profile_hook = set_axon_ntff_profile_hook
    mod.get_axon_ntff_profile_hook = get_axon_ntff_profile_hook
    sys.modules["antenv.axon_hooks"] = mod


try:
    _install_ntff_shim()
except Exception:
    pass

import concourse.bass as bass
import concourse.mybir as mybir
import concourse.tile as tile
from concourse import masks
from concourse.bass_utils import run_bass_kernel_spmd
from bass_rust import ScopedClock

# ---------------------------------------------------------------- constants
C, I, O, D, K, B = 8, 2048, 16, 16, 8, 256
NCORES = 8
IL = I // NCORES          # 256 i's per core
F32 = mybir.dt.float32
F32R = mybir.dt.float32r
BF16 = mybir.dt.bfloat16
CB = C * B
KB_ = K * B               # 2048

# ------------------------------------------------- tile tail-drain workaround
_MAX_WAITS = 1


def _patched_drain_and_barrier(self, tick_clock, wait_clock):
    nc = self.nc
    drain_inst = nc.sync.drain()
    wait_clock.add_sem_waits(
        drain_inst.ins, ScopedClock({None: tick_clock.global_clock})
    )
    si = drain_inst.ins.sync_info
    if si is not None and si.on_wait and len(si.on_wait) > _MAX_WAITS:
        waits = list(si.on_wait)
        si.on_wait = waits[:_MAX_WAITS]
        for i in range(_MAX_WAITS, len(waits), _MAX_WAITS):
            extra = nc.sync.drain()
            extra.ins.sync_info = mybir.SyncInfo(
                on_wait=waits[i : i + _MAX_WAITS], on_update=[]
            )
    nc.all_engine_barrier()
    assert self.sems is not None
    popped = nc._tile_sem_poison_stack.pop()
    assert popped is self._sem_poison
    nc.clear_and_free_semaphores(list(self.sems.allocated().values()))
    nc.all_engine_barrier()


tile.TileContext._drain_and_barrier = _patched_drain_and_barrier

_fix_ctr = [0]


def fixup_multi_waits(nc):
    """walrus in this toolchain accepts at most one sem wait per instruction;
    hoist extra waits onto same-engine drains placed just before."""
    for f in nc.m.functions:
        for bb in f.blocks:
            out = []
            for inst in bb.instructions:
                si = inst.sync_info
                if si is not None and si.on_wait and len(si.on_wait) > _MAX_WAITS:
                    waits = list(si.on_wait)
                    for i in range(0, len(waits) - _MAX_WAITS, _MAX_WAITS):
                        _fix_ctr[0] += 1
                        d = mybir.InstDrain(
                            name=f"waitsplit_{_fix_ctr[0]}", ins=[], outs=[]
                        )
                        d.engine = inst.engine
                        d.sync_info = mybir.SyncInfo(
                            on_wait=waits[i : i + _MAX_WAITS], on_update=[]
                        )
                        out.append(d)
                    si.on_wait = waits[len(waits) - _MAX_WAITS :]
                out.append(inst)
            bb.instructions[:] = out
    return nc


def build_all(fixup=True):
    nc = bass.Bass("TRN2", target_bir_lowering=False, debug=False,
                   num_devices=NCORES)
    W_d = nc.dram_tensor("W", [C, IL, O, D, K], F32, kind="ExternalInput").ap()
    x_d = nc.dram_tensor("x", [B, IL, K], F32, kind="ExternalInput").ap()
    v_d = nc.dram_tensor("v", [C, O, B], F32R, kind="ExternalOutput").ap()
    xt_d = nc.dram_tensor("xt", [IL * K, B], BF16).ap()
    HB = CB // 2  # 4 classes per collective half
    cc_in = [[nc.dram_tensor(f"cc_in{t}_{u}", [16, HB], F32).ap()
              for u in range(2)] for t in range(3)]
    cc_out = [[nc.dram_tensor(f"cc_out{t}_{u}", [16, HB], F32).ap()
               for u in range(2)] for t in range(3)]

    with tile.TileContext(nc) as tc:
        with (
            tc.tile_pool(name="const", bufs=1) as constp,
            tc.tile_pool(name="persist", bufs=1) as pers,
            tc.tile_pool(name="small", bufs=4) as smallp,
        ):
            # ---------------- constants
            ident = constp.tile([128, 128], F32)
            masks.make_identity(nc, ident[:])
            identb = constp.tile([128, 128], BF16)
            with nc.allow_low_precision(reason="identity copy"):
                nc.vector.tensor_copy(identb[:], ident[:])
            ones16f = constp.tile([16, 1], F32)
            nc.gpsimd.memset(ones16f[:], 1.0)
            ones16 = constp.tile([16, 1], F32R)
            ones1f = constp.tile([1, 16], F32)
            nc.gpsimd.memset(ones1f[:], 1.0)
            ones1 = constp.tile([1, 16], F32R)
            with nc.allow_low_precision(reason="ones copy"):
                nc.vector.tensor_copy(ones16[:], ones16f[:])
                nc.vector.tensor_copy(ones1[:], ones1f[:])

            # ---------------- persistent state
            xt_i = pers.tile([128, 2 * KB_], BF16)      # [i, (h, k, b)]
            wf = pers.tile([128, 2 * C * 128], BF16)    # [i, (c, h, (k,o))]
            T2 = pers.tile([16, 2 * C * K * 128], BF16)  # [o, (c, h, k, i)]
            w_acc = pers.tile([16, CB], BF16)
            bt = pers.tile([128, 2 * CB], BF16)         # [i, (h, c, b)]
            e_all = pers.tile([128, 2 * CB], BF16)      # exp, then ct in-place

            # ------- allreduce + squash helper, one class-half at a time so
            # the collective for classes 0-3 overlaps compute of classes 4-7
            def allreduce_squash(t, u, pre, last, sqp):
                nm = f"{t}_{u}"
                nc.gpsimd.collective_compute(
                    "AllReduce",
                    mybir.AluOpType.add,
                    replica_groups=[list(range(NCORES))],
                    ins=[cc_in[t][u].opt()],
                    outs=[cc_out[t][u].opt()],
                )
                s_sum = sqp.tile([16, HB], F32, tag="s_sum", name=f"ss{nm}")
                nc.sync.dma_start(s_sum[:], cc_out[t][u][:, :])
                sq = sqp.tile([16, HB], F32R, tag="sq", name=f"sq{nm}")
                nc.scalar.activation(
                    sq[:], s_sum[:], mybir.ActivationFunctionType.Square,
                    scale=pre,
                )
                with tc.tile_pool(name=f"sqps{nm}", bufs=1,
                                  space="PSUM") as sqps:
                    ssq_ps = sqps.tile([1, HB], F32, tag="ssq")
                    for j in range(2):
                        nc.tensor.matmul(
                            ssq_ps[:, j * 512 : (j + 1) * 512],
                            ones16[:],
                            sq[:, j * 512 : (j + 1) * 512],
                            start=True, stop=True,
                        )
                    ssq_row = sqp.tile([1, HB], F32R, tag="row_tmp",
                                       name=f"ssq_row{nm}")
                    nc.scalar.copy(ssq_row[:], ssq_ps[:])
                ssq = sqp.tile([128, 8], F32R, tag="ssq_rs", name=f"ssqr{nm}")
                nc.sync.dma_start(
                    ssq[:], ssq_row[:].rearrange("u (p f) -> u p f", p=128)
                )
                den1 = sqp.tile([128, 8], F32, tag="den1", name=f"den1{nm}")
                nc.vector.tensor_scalar_add(den1[:], ssq[:], 1.0)
                r1 = sqp.tile([128, 8], F32, tag="r1", name=f"r1{nm}")
                nc.vector.reciprocal(r1[:], den1[:])
                rt = sqp.tile([128, 8], F32, tag="rt", name=f"rt{nm}")
                nc.scalar.sqrt(rt[:], ssq[:])
                r2 = sqp.tile([128, 8], F32, tag="r2", name=f"r2{nm}")
                nc.vector.reciprocal(r2[:], rt[:])
                m1 = sqp.tile([128, 8], F32, tag="m1", name=f"m1{nm}")
                nc.vector.tensor_mul(m1[:], ssq[:], r1[:])
                scale_rs = sqp.tile([128, 8], F32R, tag="scale_rs",
                                    name=f"srs{nm}")
                nc.vector.tensor_mul(scale_rs[:], m1[:], r2[:])
                if pre != 1.0:
                    nc.vector.tensor_scalar_mul(scale_rs[:], scale_rs[:], pre)
                scale_row = sqp.tile([1, HB], F32R, tag="row_tmp",
                                     name=f"srow{nm}")
                nc.sync.dma_start(
                    scale_row[:].rearrange("u (p f) -> u p f", p=128),
                    scale_rs[:],
                )
                with tc.tile_pool(name=f"bcps{nm}", bufs=1,
                                  space="PSUM") as bcps:
                    bc_ps = bcps.tile([16, HB], F32, tag="bc")
                    for j in range(2):
                        nc.tensor.matmul(
                            bc_ps[:, j * 512 : (j + 1) * 512],
                            ones1[:],
                            scale_row[:, j * 512 : (j + 1) * 512],
                            start=True, stop=True,
                        )
                    v_sb = sqp.tile([16, HB], F32R, tag="v_sbr",
                                    name=f"vsb{nm}")
                    with nc.allow_low_precision(reason="f32r full range"):
                        nc.vector.tensor_mul(v_sb[:], s_sum[:], bc_ps[:])
                    if last:
                        for cc in range(4):
                            nc.sync.dma_start(
                                v_d[u * 4 + cc],
                                v_sb[:, cc * B : (cc + 1) * B],
                            )
                    elif t == 0:
                        with nc.allow_low_precision(reason="w bf16"):
                            nc.vector.tensor_copy(
                                w_acc[:, u * HB : (u + 1) * HB], v_sb[:]
                            )
                    else:
                        with nc.allow_low_precision(reason="w accum"):
                            nc.vector.tensor_add(
                                w_acc[:, u * HB : (u + 1) * HB],
                                w_acc[:, u * HB : (u + 1) * HB],
                                v_sb[:],
                            )

            # ---------------- phase A: x -> xt_d -> xt_i
            phio_cm = contextlib.ExitStack()
            phio = phio_cm.enter_context(tc.tile_pool(name="phio", bufs=3))
            wtree = phio_cm.enter_context(tc.tile_pool(name="wtree", bufs=2))
            with tc.tile_pool(name="xps", bufs=4, space="PSUM") as xps:
                for bh in range(2):
                    xin = phio.tile([128, IL * K], F32, tag="xin", bufs=2)
                    nc.sync.dma_start(
                        xin[:],
                        x_d[bh * 128 : (bh + 1) * 128].rearrange(
                            "b i k -> b (i k)"
                        ),
                    )
                    xc = phio.tile([128, IL * K // 128 * 128], BF16, tag="xc",
                                   bufs=1)
                    for q in range(16):
                        ps = xps.tile([128, 128], F32)
                        nc.tensor.transpose(
                            ps[:], xin[:, q * 128 : (q + 1) * 128], ident[:]
                        )
                        nc.scalar.copy(xc[:, q * 128 : (q + 1) * 128], ps[:])
                    # dst rows (q*128+p), cols [bh*128, bh*128+128)
                    nc.scalar.dma_start(
                        xt_d.rearrange("(q p) b -> p q b", p=128)[
                            :, :, bh * 128 : (bh + 1) * 128
                        ],
                        xc[:].rearrange("p (q b) -> p q b", q=16),
                    )
            for h in range(2):
                # src rows i*8+k with i = h*128+p -> per partition 4KB run
                nc.sync.dma_start(
                    xt_i[:, h * KB_ : (h + 1) * KB_],
                    xt_d[h * 1024 : (h + 1) * 1024].rearrange(
                        "(p k) b -> p (k b)", k=K
                    ),
                )

            # ---------------- phase B: W -> wf (d-reduce trees) -> T, s0
            with (
                tc.tile_pool(name="tps", bufs=2, space="PSUM") as tpsp,
                tc.tile_pool(name="s0ps", bufs=2, space="PSUM") as s0ps,
                tc.tile_pool(name="sq0", bufs=1) as sqp0,
            ):
                for c in range(C):
                    for h in range(2):
                        t = 2 * c + h
                        wt = phio.tile([128, O * D * K], F32, tag="wt", bufs=2)
                        (nc.sync if h == 0 else nc.scalar).dma_start(
                            wt[:],
                            W_d[c, h * 128 : (h + 1) * 128].rearrange(
                                "p o d k -> p (o d k)"
                            ),
                        )
                        # reduce over d in 4 levels of strided adds.  All on
                        # DVE: gpsimd must stay empty before the first
                        # collective trigger or the cross-core barrier (and
                        # with it AllReduce 0) queues behind setup work.
                        eng = nc.vector
                        v4 = wt[:].rearrange("p (o d k) -> p o d k", o=O, d=D,
                                             k=K)
                        a1 = wtree.tile([128, 1024], F32, tag="a1")
                        a1v = a1[:].rearrange("p (o d k) -> p o d k", o=O, d=8,
                                              k=K)
                        eng.tensor_add(a1v, v4[:, :, 0:8, :], v4[:, :, 8:16, :])
                        a2 = wtree.tile([128, 512], F32, tag="a2")
                        a2v = a2[:].rearrange("p (o d k) -> p o d k", o=O, d=4,
                                              k=K)
                        eng.tensor_add(a2v, a1v[:, :, 0:4, :], a1v[:, :, 4:8, :])
                        a3 = wtree.tile([128, 256], F32, tag="a3")
                        a3v = a3[:].rearrange("p (o d k) -> p o d k", o=O, d=2,
                                              k=K)
                        eng.tensor_add(a3v, a2v[:, :, 0:2, :], a2v[:, :, 2:4, :])
                        # final: f32 -> bf16, output layout (k, o): o str 1, k str 16
                        wfs = wf[:, t * 128 : (t + 1) * 128].rearrange(
                            "p (k u o) -> p o u k", k=K, u=1
                        )
                        with nc.allow_low_precision(reason="wsum bf16"):
                            eng.tensor_add(
                                wfs, a3v[:, :, 0:1, :], a3v[:, :, 1:2, :]
                            )
                        # transpose each k-slice [128,16] -> [16,128] (base 0)
                        tp = tpsp.tile([16, K * 128], BF16, tag="tp")
                        for k in range(K):
                            nc.tensor.transpose(
                                tp[:, k * 128 : (k + 1) * 128],
                                wf[:, t * 128 + k * 16 : t * 128 + (k + 1) * 16],
                                identb[:],
                            )
                        nc.scalar.copy(
                            T2[:, t * K * 128 : (t + 1) * K * 128], tp[:]
                        )
                    # s0: uniform-c iteration 0 partials
                    s0p = s0ps.tile([16, B], F32, tag="s0p")
                    for h in range(2):
                        t = 2 * c + h
                        for k in range(K):
                            nc.tensor.matmul(
                                s0p[:],
                                wf[:, t * 128 + k * 16 : t * 128 + (k + 1) * 16],
                                xt_i[:, h * KB_ + k * B : h * KB_ + (k + 1) * B],
                                start=(h == 0 and k == 0),
                                stop=(h == 1 and k == K - 1),
                            )
                    s0sb = smallp.tile([16, B], F32, tag="s_sb",
                                       name=f"s0sb{c}")
                    nc.scalar.copy(s0sb[:], s0p[:])
                    nc.sync.dma_start(
                        cc_in[0][c // 4][:, (c % 4) * B : (c % 4 + 1) * B],
                        s0sb[:],
                    )
                    if c == 3:
                        allreduce_squash(0, 0, 1.0 / C, last=False, sqp=sqp0)
                    elif c == 7:
                        allreduce_squash(0, 1, 1.0 / C, last=False, sqp=sqp0)

            phio_cm.close()

            # ---------------- routing iterations 1 and 2
            # DVE per-instruction overhead is ~0.4us, so everything below
            # works on the largest slices SBUF allows.
            with (
                tc.tile_pool(name="workp", bufs=1) as workp,
                tc.tile_pool(name="p2p", bufs=2) as p2p,
                tc.tile_pool(name="fldp", bufs=1) as fldp,
                tc.tile_pool(name="softp", bufs=1) as softp,
            ):
                for it in range(1, 3):
                    # ---- phase 1: bt[i, (h, c, b)] = sum_k xt*G
                    with tc.tile_pool(name=f"gps{it}", bufs=2,
                                      space="PSUM") as gps:
                        for h in range(2):
                            for cg in range(4):  # c-pairs
                                p2 = p2p.tile([128, 2 * KB_], BF16, tag="p2")
                                for cc in range(2):
                                    c = cg * 2 + cc
                                    t = 2 * c + h
                                    g = gps.tile([128, KB_], F32, tag="g")
                                    for k in range(K):
                                        nc.tensor.matmul(
                                            g[:, k * B : (k + 1) * B],
                                            T2[:, (t * K + k) * 128 :
                                               (t * K + k + 1) * 128],
                                            w_acc[:, c * B : (c + 1) * B],
                                            start=True, stop=True,
                                        )
                                    # Pool can't read PSUM: Act narrows G to
                                    # bf16 SBUF, then DVE/Pool go all-bf16.
                                    g16 = workp.tile([128, KB_], BF16,
                                                     tag="g16", bufs=3)
                                    nc.scalar.copy(g16[:], g[:])
                                    peng = (nc.gpsimd if t % 4 == 1
                                            else nc.vector)
                                    with nc.allow_low_precision(reason="P"):
                                        peng.tensor_mul(
                                            p2[:, cc * KB_ : (cc + 1) * KB_],
                                            xt_i[:, h * KB_ : (h + 1) * KB_],
                                            g16[:],
                                        )
                                # fold over k: 3 big strided adds per c-pair
                                p2v = p2[:].rearrange(
                                    "p (c k b) -> p c k b", c=2, k=K
                                )
                                fl1 = fldp.tile([128, KB_], BF16, tag="fl1")
                                f1v = fl1[:].rearrange(
                                    "p (c k b) -> p c k b", c=2, k=4
                                )
                                fl2 = fldp.tile([128, KB_ // 2], BF16,
                                                tag="fl2")
                                f2v = fl2[:].rearrange(
                                    "p (c k b) -> p c k b", c=2, k=2
                                )
                                btv = bt[:, h * CB + cg * 2 * B :
                                         h * CB + (cg * 2 + 2) * B] \
                                    .rearrange("p (c u b) -> p c u b",
                                               c=2, u=1)
                                with nc.allow_low_precision(reason="fold"):
                                    nc.vector.tensor_add(
                                        f1v, p2v[:, :, 0:4, :],
                                        p2v[:, :, 4:8, :]
                                    )
                                    nc.vector.tensor_add(
                                        f2v, f1v[:, :, 0:2, :],
                                        f1v[:, :, 2:4, :]
                                    )
                                    nc.vector.tensor_add(
                                        btv, f2v[:, :, 0:1, :],
                                        f2v[:, :, 1:2, :]
                                    )

                    # ---- phase 2: softmax over classes, both halves at
                    # once.  Iteration 1 logits are bounded by ~60 (|v0|<1),
                    # so exp() is safe without the max-subtraction there.
                    btv = bt[:].rearrange("p (h c b) -> p h c b", h=2, c=C)
                    m1 = softp.tile([128, 2 * 4 * B], BF16, tag="m1")
                    m1v = m1[:].rearrange("p (h c b) -> p h c b", h=2, c=4)
                    m2 = softp.tile([128, 2 * 2 * B], BF16, tag="m2")
                    m2v = m2[:].rearrange("p (h c b) -> p h c b", h=2, c=2)
                    rmax = softp.tile([128, 2 * B], BF16, tag="rmax")
                    rmv = rmax[:].rearrange("p (h u b) -> p h u b", h=2, u=1)
                    sub = softp.tile([128, 2 * CB], BF16, tag="sub")
                    subv = sub[:].rearrange("p (h c b) -> p h c b", h=2, c=C)
                    if True:
                        with nc.allow_low_precision(reason="softmax max"):
                            nc.vector.tensor_max(
                                m1v, btv[:, :, 0:4, :], btv[:, :, 4:8, :]
                            )
                            nc.vector.tensor_max(
                                m2v, m1v[:, :, 0:2, :], m1v[:, :, 2:4, :]
                            )
                            nc.vector.tensor_max(
                                rmv, m2v[:, :, 0:1, :], m2v[:, :, 1:2, :]
                            )
                            nc.vector.tensor_sub(
                                subv, btv,
                                rmax[:].rearrange("p (h b) -> p h b", h=2)
                                .unsqueeze(2).broadcast_to([128, 2, C, B]),
                            )
                        nc.scalar.activation(
                            e_all[:], sub[:],
                            mybir.ActivationFunctionType.Exp,
                        )
                    ev = e_all[:].rearrange("p (h c b) -> p h c b", h=2, c=C)
                    d1 = softp.tile([128, 2 * 4 * B], BF16, tag="m1",
                                    name=f"d1_{it}")
                    d1v = d1[:].rearrange("p (h c b) -> p h c b", h=2, c=4)
                    d2 = softp.tile([128, 2 * 2 * B], BF16, tag="m2",
                                    name=f"d2_{it}")
                    d2v = d2[:].rearrange("p (h c b) -> p h c b", h=2, c=2)
                    den = softp.tile([128, 2 * B], F32, tag="den")
                    denv = den[:].rearrange("p (h u b) -> p h u b", h=2, u=1)
                    with nc.allow_low_precision(reason="den partials bf16"):
                        nc.vector.tensor_add(
                            d1v, ev[:, :, 0:4, :], ev[:, :, 4:8, :]
                        )
                        nc.vector.tensor_add(
                            d2v, d1v[:, :, 0:2, :], d1v[:, :, 2:4, :]
                        )
                    nc.vector.tensor_add(
                        denv, d2v[:, :, 0:1, :], d2v[:, :, 1:2, :]
                    )
                    rec = softp.tile([128, 2 * B], F32, tag="rec")
                    nc.vector.reciprocal(rec[:], den[:])
                    recb = softp.tile([128, 2 * B], BF16, tag="recb")
                    with nc.allow_low_precision(reason="rec bf16"):
                        nc.vector.tensor_copy(recb[:], rec[:])
                        # ct overwrites e in place
                        nc.vector.tensor_mul(
                            ev, ev,
                            recb[:].rearrange("p (h b) -> p h b", h=2)
                            .unsqueeze(2).broadcast_to([128, 2, C, B]),
                        )

                    # ---- phase 3: y = ct*x, s_c = sum_{h,k} wf^T y
                    with (
                        tc.tile_pool(name=f"sps{it}", bufs=1,
                                     space="PSUM") as sps,
                        tc.tile_pool(name=f"sq{it}", bufs=1) as sqpi,
                    ):
                        s_ps = sps.tile([16, CB], F32, tag="s")
                        for c in range(C):
                            y = workp.tile([128, 2 * KB_], BF16, tag="y",
                                           bufs=2)
                            yeng = nc.gpsimd if (c % 4) == 3 else nc.vector
                            with nc.allow_low_precision(reason="y bf16"):
                                yeng.tensor_mul(
                                    y[:].rearrange("p (h k b) -> p h k b",
                                                   h=2, k=K),
                                    xt_i[:].rearrange("p (h k b) -> p h k b",
                                                      h=2, k=K),
                                    e_all[:].rearrange(
                                        "p (h c b) -> p h c b", h=2, c=C
                                    )[:, :, c : c + 1, :]
                                    .broadcast_to([128, 2, K, B]),
                                )
                            for h in range(2):
                                t = 2 * c + h
                                for k in range(K):
                                    nc.tensor.matmul(
                                        s_ps[:, c * B : (c + 1) * B],
                                        wf[:, t * 128 + k * 16 :
                                           t * 128 + (k + 1) * 16],
                                        y[:, h * KB_ + k * B :
                                          h * KB_ + (k + 1) * B],
                                        start=(h == 0 and k == 0),
                                        stop=(h == 1 and k == K - 1),
                                    )
                            if c == 3 or c == 7:
                                u = c // 4
                                s_sb = smallp.tile(
                                    [16, HB], F32, tag="s_sbf",
                                    name=f"s_sb{it}_{u}", bufs=2,
                                )
                                nc.scalar.copy(
                                    s_sb[:], s_ps[:, u * HB : (u + 1) * HB]
                                )
                                nc.sync.dma_start(cc_in[it][u][:, :], s_sb[:])
                                allreduce_squash(
                                    it, u, 1.0, last=(it == 2), sqp=sqpi
                                )
    return fixup_multi_waits(nc) if fixup else nc


_NC = None


def kernel(x: np.ndarray, W: np.ndarray, _timings=None) -> np.ndarray:
    global _NC
    x = np.asarray(x, np.float32)
    W = np.asarray(W, np.float32)
    if _NC is None:
        _NC = build_all()
    in_maps = []
    for j in range(NCORES):
        sl = slice(j * IL, (j + 1) * IL)
        in_maps.append(
            {
                "W": np.ascontiguousarray(W[:, 0, sl]),
                "x": np.ascontiguousarray(x[:, sl, :]),
            }
        )
    res = run_bass_kernel_spmd(
        _NC, in_maps, core_ids=list(range(NCORES)),
        trace=_timings is not None,
    )
    if _timings is not None:
        _timings.append(res.exec_time_ns)
    v = res.results[0]["v"].astype(np.float32)  # [C, O, B]
    return np.ascontiguousarray(v.transpose(2, 0, 1))
